# revision 58
# baseline (speedup 1.0000x reference)
"""Multi-head graph attention (GAT-style) Trainium2 Bass kernel.

Full-input contract: kernel(**inputs) takes the complete arrays, shards
batch-wise across 8 NeuronCores (2 batches each), and gathers the output.

Math per batch b, head h (KD=16 head dim):
  Q = h @ Wq_h, K = h @ Wk_h, V = h @ Wv_h            [N, 16]
  compatT[m, n] = (K Q^T)[m, n]                        [N, N] (transposed)
  p = exp(0.25 * compatT) * adjT                       (mask after exp; exact:
      masked entries are exactly 0, matching softmax(-inf) * adj)
  headsT[v, n] = (V'.T @ p)  with V' = [V | 1 | 0pad]  -> row 16 = denominator
  out[n, :] = sum_h (headsT_h / den_h).T @ Wout_h + h[n, :]

Design: ACT's exp stream (128 calls x ~1.0us back-to-back) is the pacing
engine; every other engine hides under it.
 - Heads live in 32-partition bands (head h -> quad h//4, band h%4).
   Banded zero-padded projection weights put q/k of band b at SBUF
   partitions [32b, 32b+16), so the K=16 compat matmuls of a head PAIR
   run concurrently via PE row tiling (tile_position=(32b, 0)).
 - Each pair's two [128,512] compat tiles share one 2-bank PSUM buffer;
   one exp call covers both. Pair buffers rotate 3-deep (banks 0-5);
   PV accumulators for the two quads hold banks 6-7 per n-half.  The
   mask multiply runs one DVE op per pair (adjT operand repeated via a
   stride-0 AP); PV matmuls lag PV_LAG pairs behind compat in PE
   program order so the exp <- mask <- exp cross-engine cycle has
   several exp-periods of budget and DVE jitter never stalls ACT.
 - Mid-stream denominators: the 4 den rows (partition 32c+16) of each
   quad's PV accumulator are reshaped by gpsimd SBUF->SBUF DMAs into
   [128,16] so the exact DVE reciprocal uses all lanes (~250ns; the
   8-cyc/elem iterative divide makes a [4,512] recip 13x slower),
   scattered back to a [4,512] row tile, and band-broadcast by a
   selector matmul (sel.T @ rec4) on the mostly-idle PE.  Epilogue
   stages are software-pipelined INTO the next n-half's stream, spaced
   so only one stage borrows a pair PSUM buffer at a time (every borrow
   shrinks the compat rotation for one step, ~0.5us) and every consumer
   fires >=2 steps after its cross-engine producer (in-order queues: a
   too-early consumer head-of-line-blocks its whole engine).
 - Final-segment tail: pairs run quad-1-first so both quad chains
   overlap; quad 1's hu copy and both reciprocals ride the idle ScalarE
   as exp(-ln d) -- Ln/Exp share one ACT table set (natural_log_exp) so
   there is no table switch and no 8-cyc/elem DVE iterative divide.
 - A dummy 1-element exp at t=0 forces the ACT table load during the
   NEFF preamble.  First-batch DMAs are minimal and priority-ordered
   (wk0|wq0 block, then hT halves on two queues, then adj chunk 0); the
   bulk adj/h transfers are deferred into the stream so they don't
   steal HBM bandwidth from the critical path.  Quad 1's nt=0
   projections burst into steps 0-1 so the first compat+exp aren't
   queued behind them on the PE.
 - Every SBUF tile has exactly ONE writer DMA (wpk/hT/adj arrive in
   per-chunk tiles): a tile with two writer DMAs mis-attributes reads
   emitted close to the transfers and reads garbage.
 - All tiles live in two pools (one SBUF, one PSUM, per-tag buf counts)
   -- every pool costs a full cross-engine semaphore barrier mesh at
   teardown (~10us for an 11-pool layout).
 - adj / h ship in partition-contiguous host layouts (one descriptor
   per partition); adjT as bf16 (0/1 exact), h/q/k/v in bf16 (rel err
   ~3e-3 vs the 2e-2 gate).
"""

import os
import numpy as np
import ml_dtypes
from contextlib import ExitStack

import concourse.bass as bass
import concourse.mybir as mybir
import concourse.tile as tile
from concourse.bass_utils import run_bass_kernel_spmd

B, N, E, H, KD = 16, 1024, 128, 8, 16
CORES = 8
BPC = B // CORES  # batches per core
F32 = mybir.dt.float32
BF16 = mybir.dt.bfloat16
NT = 512  # n-half width (one PSUM bank of fp32 per [128, NT] tile)
MC = N // 128  # number of 128-row chunks of m
VP = 32  # padded per-head V columns (16 vals + 1 ones + 15 zeros)
NR = NT // 32  # free elems per lane in the [128, 2, NR] reciprocal
PV_LAG = 4


def build_kernel():
    nc = bass.Bass()
    hT_d = nc.dram_tensor("ht", [BPC, E, N], BF16, kind="ExternalInput")
    h_d = nc.dram_tensor("hn", [BPC, 128, MC, E], F32, kind="ExternalInput")
    adjt_d = nc.dram_tensor("adjt", [BPC, 128, MC, N], BF16, kind="ExternalInput")
    # packed weights: [wk0|wq0|wk1|wq1|wv|wo0|wo1|sel4pad] as [128, 8*128]
    wpk_d = nc.dram_tensor("wpk", [128, 8 * 128], BF16, kind="ExternalInput")
    sel_d = nc.dram_tensor("sel", [4, 128], BF16, kind="ExternalInput")
    out_d = nc.dram_tensor("out", [BPC, N, E], F32, kind="ExternalOutput")
    DEBUG = bool(int(os.environ.get("KDBG", "0")))
    if DEBUG:
        dbg_hu = nc.dram_tensor("dbg_hu", [2, 128, NT], F32, kind="ExternalOutput")
        dbg_d = nc.dram_tensor("dbg_d", [2, 128, NR], F32, kind="ExternalOutput")
        dbg_r = nc.dram_tensor("dbg_r", [2, 128, NR], BF16, kind="ExternalOutput")
        dbg_rec = nc.dram_tensor("dbg_rec", [2, 4, NT], BF16, kind="ExternalOutput")
        dbg_pm = nc.dram_tensor("dbg_pm", [128, 2 * NT], BF16, kind="ExternalOutput")

    with ExitStack() as ctx:
        tc = ctx.enter_context(tile.TileContext(nc))
        sb = ctx.enter_context(tc.tile_pool(name="sb", bufs=2))
        ps = ctx.enter_context(tc.tile_pool(name="ps", bufs=1, space="PSUM"))

        # packed weights land in two tiles so each has exactly ONE writer
        # DMA -- a tile with two writer DMAs mis-attributes reads emitted
        # close to the transfers (the b0 projection units) and reads
        # garbage.  wpk_a is the 64KB critical block (wk0|wq0).
        wpk_a = sb.tile([128, 256], BF16, tag="wpk_a", bufs=1)
        wpk_b = sb.tile([128, 6 * 128], BF16, tag="wpk_b", bufs=1)
        wk_q = [wpk_a[:, 0:128], wpk_b[:, 0:128]]
        wq_q = [wpk_a[:, 128:256], wpk_b[:, 128:256]]
        wv_sb = wpk_b[:, 256:384]
        wo_q = [wpk_b[:, 384:512], wpk_b[:, 512:640]]
        sel4_sb = wpk_b[:, 640:644]  # [128, 4]: ones at (32c+16, c)

        # band-broadcast selector: sel[c, 32c+j] = 1 (j < 17) so
        # (sel.T @ rec4) replicates rec4 row c into band c's rows on PE.
        sel_sb = sb.tile([4, 128], BF16, tag="sel", bufs=1)

        # dummy activation at t=0: walrus emits the ACT table load right
        # before it, so the ~2.7us load hides under the NEFF preamble.
        scratch = sb.tile([1, 8], F32, tag="scr", bufs=1)
        scratch_o = sb.tile([1, 8], BF16, tag="scro", bufs=1)
        nc.vector.memset(scratch, 0.0)
        nc.scalar.activation(
            out=scratch_o,
            in_=scratch,
            func=mybir.ActivationFunctionType.Exp,
            scale=0.25,
        )

        ios = {}

        def prefetch(b, first=False):
            if first:
                # priority-ordered critical set on two queues: wk0|wq0
                # block + hT halves + adj chunk 0.  The bulk transfers
                # (adj chunks 1-7, h) are deferred into the stream
                # (deferred_b0) so they don't steal HBM bandwidth from
                # the critical path.  Every tile has exactly one writer.
                hT_a = sb.tile([E, NT], BF16, tag="ht_a", bufs=1, name="hta")
                hT_b = sb.tile([E, NT], BF16, tag="ht_b", bufs=1, name="htb")
                adj_a = sb.tile([128, 1, N], BF16, tag="adj_a", bufs=1, name="aja")
                adj_b1 = sb.tile([128, 3, N], BF16, tag="adj_b1", bufs=1, name="ab1")
                adj_b2 = sb.tile(
                    [128, MC - 4, N], BF16, tag="adj_b2", bufs=1, name="ab2"
                )
                h_sb = sb.tile([128, MC, E], F32, tag="hn", name="hns")
                nc.gpsimd.dma_start(out=wpk_a, in_=wpk_d[:, 0:256])
                nc.sync.dma_start(out=hT_a, in_=hT_d[b, :, 0:NT])
                nc.gpsimd.dma_start(out=wpk_b, in_=wpk_d[:, 256:])
                nc.gpsimd.dma_start(out=sel_sb, in_=sel_d[:, :])
                nc.gpsimd.dma_start(out=hT_b, in_=hT_d[b, :, NT:N])
                nc.sync.dma_start(out=adj_a, in_=adjt_d[b, :, 0:1, :])
                ios[b] = {"ht": [hT_a, hT_b], "adj": [adj_a, adj_b1, adj_b2],
                          "h": h_sb}
            else:
                hT_sb = sb.tile([E, N], BF16, tag="ht", bufs=1, name="hts")
                adjT_sb = sb.tile([128, MC, N], BF16, tag="adj", bufs=1, name="adjs")
                h_sb = sb.tile([128, MC, E], F32, tag="hn", name="hns")
                nc.sync.dma_start(out=hT_sb, in_=hT_d[b, :, :])
                nc.sync.dma_start(out=adjT_sb, in_=adjt_d[b])
                nc.sync.dma_start(out=h_sb, in_=h_d[b])
                ios[b] = {"ht": [hT_sb], "adj": [adjT_sb], "h": h_sb}

        def ht_slice(b, c0, c1):
            parts = ios[b]["ht"]
            if len(parts) == 1:
                return parts[0][:, c0:c1]
            if c1 <= NT:
                return parts[0][:, c0:c1]
            return parts[1][:, c0 - NT : c1 - NT]

        def adj_of(b, mc, nt):
            parts = ios[b]["adj"]
            if len(parts) == 1:
                return parts[0][:, mc, nt * NT : (nt + 1) * NT]
            if mc == 0:
                return parts[0][:, 0, nt * NT : (nt + 1) * NT]
            if mc < 4:
                return parts[1][:, mc - 1, nt * NT : (nt + 1) * NT]
            return parts[2][:, mc - 4, nt * NT : (nt + 1) * NT]

        def deferred_b0(step):
            # bulk transfers for batch 0, after the critical DMAs
            if step == 0:
                nc.sync.dma_start(out=ios[0]["adj"][1], in_=adjt_d[0, :, 1:4, :])
            elif step == 1:
                nc.sync.dma_start(out=ios[0]["adj"][2], in_=adjt_d[0, :, 4:MC, :])
            elif step == 2:
                nc.sync.dma_start(out=ios[0]["h"], in_=h_d[0])

        bands = {}

        def make_prologue_units(b, split_first=False):
            """Projection + V-build for batch b as coarse closures.  Full
            units do both n-halves of a q/k row pair in one pair-buffer
            borrow; V-mega units build 4 m-chunks of V' at once.  For batch
            0 the first four units are nt=0-only so the first compat can
            issue as soon as the first hT half lands."""
            qb = [sb.tile([128, N], BF16, tag=f"q{q}", name=f"qb{q}") for q in range(2)]
            kb = [sb.tile([128, N], BF16, tag=f"k{q}", name=f"kb{q}") for q in range(2)]
            v_nat = [
                sb.tile([128, H, VP], BF16, tag=f"v{mc}", name=f"v{mc}")
                for mc in range(MC)
            ]
            bands[b] = (qb, kb, v_nat)

            def proj_half(w_sb, dst, nt):
                def run():
                    pp = ps.tile([128, NT], F32, tag="pair", bufs=3, name="pp")
                    nc.tensor.matmul(
                        out=pp,
                        lhsT=w_sb,
                        rhs=ht_slice(b, nt * NT, (nt + 1) * NT),
                        start=True,
                        stop=True,
                    )
                    nc.vector.tensor_copy(out=dst[:, nt * NT : (nt + 1) * NT], in_=pp)

                return run

            def proj_full(w_sb, dst):
                def run():
                    pp = ps.tile([128, 2, NT], F32, tag="pair", bufs=3, name="pp")
                    for nt in range(2):
                        nc.tensor.matmul(
                            out=pp[:, nt, :],
                            lhsT=w_sb,
                            rhs=ht_slice(b, nt * NT, (nt + 1) * NT),
                            start=True,
                            stop=True,
                        )
                    nc.vector.tensor_copy(
                        out=dst.rearrange("p (t n) -> p t n", t=2), in_=pp
                    )

                return run

            def v_unit(mc):
                def run():
                    vp = ps.tile([128, H * KD], F32, tag="pair", bufs=3, name="vp")
                    nc.tensor.matmul(
                        out=vp,
                        lhsT=ht_slice(b, mc * 128, (mc + 1) * 128),
                        rhs=wv_sb,
                        start=True,
                        stop=True,
                    )
                    vt = v_nat[mc]
                    nc.vector.tensor_copy(
                        out=vt[:, :, 0:KD],
                        in_=vp.rearrange("p (h k) -> p h k", k=KD),
                    )
                    nc.vector.memset(vt[:, :, KD : KD + 1], 1.0)
                    nc.vector.memset(vt[:, :, KD + 1 : VP], 0.0)

                return run

            if split_first:
                # pre-stream: only quad 0's nt=0 projections, so the first
                # compat+exp aren't queued behind quad 1's (whose wpk_b
                # weights land later).  k1h/v0/q1h burst into steps 0-1.
                first = [
                    proj_half(wk_q[0], kb[0], 0),
                    proj_half(wq_q[0], qb[0], 0),
                    v_unit(0),
                ]
                rest = [
                    proj_half(wk_q[1], kb[1], 0),
                    proj_half(wq_q[1], qb[1], 0),
                    v_unit(1),
                    v_unit(2),
                    v_unit(3),
                    proj_half(wk_q[0], kb[0], 1),
                    proj_half(wk_q[1], kb[1], 1),
                    v_unit(4),
                    v_unit(5),
                    v_unit(6),
                    v_unit(7),
                    proj_half(wq_q[0], qb[0], 1),
                    proj_half(wq_q[1], qb[1], 1),
                ]
                return first, rest
            return None, [
                proj_half(wk_q[0], kb[0], 0),
                proj_half(wq_q[0], qb[0], 0),
                proj_half(wk_q[1], kb[1], 0),
                proj_half(wq_q[1], qb[1], 0),
                v_unit(0),
                v_unit(1),
                v_unit(2),
                v_unit(3),
                proj_half(wk_q[0], kb[0], 1),
                proj_half(wk_q[1], kb[1], 1),
                v_unit(4),
                v_unit(5),
                v_unit(6),
                v_unit(7),
                proj_half(wq_q[0], qb[0], 1),
                proj_half(wq_q[1], qb[1], 1),
            ]

        def make_epilogue(b, nt, hp):
            """Normalize + W_out + residual for (b, nt) as staged closures.
            Stages are spaced so at most one borrows a pair PSUM buffer at
            a time, and every consumer fires >=2 steps after its producer."""
            h_sb = ios[b]["h"]
            hus, hn2, oms = {}, {}, {}
            st = {}

            dbg = DEBUG and b == 0 and nt == 0

            def hu_q(q):
                def run():
                    hu = sb.tile([128, NT], F32, tag=f"hu{q}", name=f"hu{q}")
                    nc.vector.tensor_copy(out=hu, in_=hp[q])
                    hus[q] = hu
                    if dbg:
                        nc.sync.dma_start(out=dbg_hu[q, :, :], in_=hu)
                return run

            def gather_q(q):
                def run():
                    d128 = sb.tile([128, NR], F32, tag=f"d128{q}", name=f"d1{q}")
                    for c in range(4):
                        srcp = hus[q][32 * c + KD : 32 * c + KD + 1, :]
                        src_r = bass.AP(
                            tensor=srcp.tensor,
                            offset=srcp.offset,
                            ap=[list(srcp.ap[0]), [NR, 32], [1, NR]],
                        )
                        nc.gpsimd.dma_start(
                            out=d128[32 * c : 32 * c + 32, :], in_=src_r
                        )
                    st[f"d128{q}"] = d128
                    if dbg:
                        nc.sync.dma_start(out=dbg_d[q, :, :], in_=d128)
                return run

            def recip_q(q):
                def run():
                    r128b = sb.tile([128, NR], BF16, tag=f"r128b{q}", name=f"rb{q}")
                    with nc.allow_low_precision(
                        reason="denominator recip to bf16 is within gate"
                    ):
                        nc.vector.reciprocal(out=r128b, in_=st[f"d128{q}"])
                    st[f"r128b{q}"] = r128b
                    if dbg:
                        nc.sync.dma_start(out=dbg_r[q, :, :], in_=r128b)
                return run

            def scatter_both():
                for q in range(2):
                    rec4 = sb.tile([4, NT], BF16, tag=f"rec4{q}", name=f"rc{q}")
                    dst = rec4[:, :]
                    dst_r = bass.AP(
                        tensor=dst.tensor,
                        offset=dst.offset,
                        ap=[list(dst.ap[0]), [NR, 32], [1, NR]],
                    )
                    nc.gpsimd.dma_start(out=dst_r, in_=st[f"r128b{q}"])
                    st[f"rec4{q}"] = rec4
                    if dbg:
                        nc.sync.dma_start(out=dbg_rec[q, :, :], in_=rec4)

            def selmm_both():
                bc_ps = ps.tile([128, 2, NT], F32, tag="pair", bufs=3, name="bc")
                for q in range(2):
                    nc.tensor.matmul(
                        out=bc_ps[:, q, :],
                        lhsT=sel_sb,
                        rhs=st[f"rec4{q}"],
                        start=True,
                        stop=True,
                    )
                st["bc"] = bc_ps

            def norm_q(q):
                def run():
                    hn = sb.tile([128, NT], BF16, tag=f"hn{q}", name=f"hn{q}")
                    nc.vector.tensor_mul(hn, hus[q], st["bc"][:, q, :])
                    hn2[q] = hn
                return run

            def out_mm_all():
                o_ps = ps.tile([128, 4, E], F32, tag="pair", bufs=3, name="op")
                for cl in range(4):
                    for q in range(2):
                        nc.tensor.matmul(
                            out=o_ps[:, cl, :],
                            lhsT=hn2[q][:, cl * 128 : (cl + 1) * 128],
                            rhs=wo_q[q],
                            start=(q == 0),
                            stop=(q == 1),
                        )
                st["o_ps"] = o_ps

            def out_fin_all():
                cc = nt * (NT // 128)
                ob = sb.tile([128, 4, E], F32, tag="ob", name="ob")
                nc.vector.tensor_add(ob, st["o_ps"], h_sb[:, cc : cc + 4, :])
                nc.sync.dma_start(
                    out=out_d[b, cc * 128 : (cc + 4) * 128, :].rearrange(
                        "(c p) e -> p c e", p=128
                    ),
                    in_=ob,
                )

            # spacing: at most one pair-PSUM borrow live at a time (the
            # merged bc until both norms, the merged o_ps until out_fin),
            # and every cross-engine consumer fires >=2 steps after its
            # producer so in-order queues never head-of-line block.
            stages = [
                (1, hu_q(0)),
                (3, hu_q(1)),
                (5, gather_q(0)),
                (7, gather_q(1)),
                (9, recip_q(0)),
                (10, recip_q(1)),
                (11, scatter_both),
                (13, selmm_both),
                (15, norm_q(0)),
                (17, norm_q(1)),
                (19, out_mm_all),
                (22, out_fin_all),
            ]
            return stages

        def make_tail_epilogue(b, nt, hp):
            """Latency-optimized final chain.  Quads finish in order 1 then
            0 (the final segment streams pairs 2,3,0,1).  Quad 1's hu copy
            rides the ScalarE (idle once exps end); reciprocals run as
            exp(-ln d) on ScalarE -- Ln and Exp share one ACT table set
            (natural_log_exp) so there is no table switch and the
            8-cyc/elem DVE iterative divide is avoided."""
            h_sb = ios[b]["h"]
            hus, den_ps, rec4s, rec_ps, hn2 = {}, {}, {}, {}, {}

            def hu_q(q, eng):
                def run():
                    hu = sb.tile([128, NT], BF16, tag=f"hut{q}", name=f"hut{q}")
                    if eng == "scalar":
                        nc.scalar.copy(out=hu, in_=hp[q])
                    else:
                        nc.vector.tensor_copy(out=hu, in_=hp[q])
                    hus[q] = hu
                return run

            def ext_q(q):
                def run():
                    d4 = ps.tile([4, NT], F32, tag="pair", bufs=3, name="d4")
                    nc.tensor.matmul(
                        out=d4, lhsT=sel4_sb, rhs=hus[q], start=True, stop=True
                    )
                    den_ps[q] = d4
                return run

            def recip_q(q):
                def run():
                    lnd = sb.tile([4, NT], F32, tag=f"lnd{q}", name=f"ln{q}")
                    nc.scalar.activation(
                        out=lnd,
                        in_=den_ps[q],
                        func=mybir.ActivationFunctionType.Ln,
                    )
                    rec4 = sb.tile([4, NT], BF16, tag=f"rect{q}", name=f"rt{q}")
                    with nc.allow_low_precision(
                        reason="denominator recip to bf16 is within gate"
                    ):
                        nc.scalar.activation(
                            out=rec4,
                            in_=lnd,
                            func=mybir.ActivationFunctionType.Exp,
                            scale=-1.0,
                        )
                    rec4s[q] = rec4
                return run

            def selmm_q(q):
                def run():
                    bc_ps = ps.tile([128, NT], F32, tag="pair", bufs=3, name="bc")
                    nc.tensor.matmul(
                        out=bc_ps, lhsT=sel_sb, rhs=rec4s[q], start=True, stop=True
                    )
                    rec_ps[q] = bc_ps
                return run

            def norm_q(q):
                def run():
                    hn = sb.tile([128, NT], BF16, tag=f"hn{q}", name=f"hn{q}")
                    nc.vector.tensor_mul(hn, hus[q], rec_ps[q])
                    hn2[q] = hn
                return run

            def out_cp(cp):
                def run():
                    cc = nt * (NT // 128) + cp * 2
                    o_ps = ps.tile([128, 2, E], F32, tag="pair", bufs=3, name="op")
                    for cl in range(2):
                        for q in range(2):
                            nc.tensor.matmul(
                                out=o_ps[:, cl, :],
                                lhsT=hn2[q][
                                    :,
                                    (cp * 2 + cl) * 128 : (cp * 2 + cl + 1) * 128,
                                ],
                                rhs=wo_q[q],
                                start=(q == 0),
                                stop=(q == 1),
                            )
                    ob = sb.tile([128, 2, E], F32, tag="ob", name="ob")
                    nc.vector.tensor_add(ob, o_ps, h_sb[:, cc : cc + 2, :])
                    nc.sync.dma_start(
                        out=out_d[b, cc * 128 : (cc + 2) * 128, :].rearrange(
                            "(c p) e -> p c e", p=128
                        ),
                        in_=ob,
                    )
                return run

            return {
                "hu": {q: hu_q(q, "scalar" if q == 1 else "vector") for q in range(2)},
                "ext": {q: ext_q(q) for q in range(2)},
                "recip": {q: recip_q(q) for q in range(2)},
                "selmm": {q: selmm_q(q) for q in range(2)},
                "norm": {q: norm_q(q) for q in range(2)},
                "out": [out_cp(0), out_cp(1)],
            }

        # ---- main stream over (batch, n-half) segments ----
        prefetch(0, first=True)
        b0_first, b0_rest = make_prologue_units(0, split_first=True)
        for u in b0_first:
            u()
        pending = None
        prologue_units = b0_rest
        unit_next = [0]
        units_fired = [0]
        unit_spacing = 2
        tail = None
        carry = []  # last 2 PVs of the previous segment (masks not yet done)
        for b in range(BPC):
            qb, kb, v_nat = bands.pop(b)
            for nt in range(N // NT):
                is_final = b == BPC - 1 and nt == N // NT - 1
                if nt == 1 and b + 1 < BPC:
                    prologue_units = (
                        prologue_units + make_prologue_units(b + 1)[1]
                    )
                    unit_spacing = 3
                hp = [
                    ps.tile([128, NT], F32, tag=f"hp{q}", bufs=1, name=f"hp{q}")
                    for q in range(2)
                ]
                if is_final:
                    tail = make_tail_epilogue(b, nt, hp)
                    pair_order = [2, 3, 0, 1]  # quad 1 finishes first
                else:
                    epi = make_epilogue(b, nt, hp)
                    pair_order = [0, 1, 2, 3]
                pv_queue = []

                def emit_pv(pm, mc, pair, hp=hp, v_nat=v_nat):
                    # hp/v_nat bound by VALUE: carried entries fire in the
                    # next segment, after the loop variables move on.
                    for j in range(2):
                        hh = pair * 2 + j
                        c = hh % 4
                        nc.tensor.matmul(
                            out=hp[hh // 4][32 * c : 32 * c + VP, :],
                            lhsT=v_nat[mc][:, hh, :],
                            rhs=pm[:, j * NT : (j + 1) * NT],
                            start=(mc == 0),
                            stop=(mc == MC - 1),
                            tile_position=(0, 32 * c),
                        )

                step = 0
                for mc in range(MC):
                    for pair in pair_order:
                        quad, b0 = pair // 2, (pair * 2) % 4
                        cps = ps.tile(
                            [128, 2 * NT], F32, tag="pair", bufs=3, name="cp"
                        )
                        for j in range(2):
                            bd = b0 + j
                            nc.tensor.matmul(
                                out=cps[:, j * NT : (j + 1) * NT],
                                lhsT=kb[quad][
                                    32 * bd : 32 * bd + KD,
                                    mc * 128 : (mc + 1) * 128,
                                ],
                                rhs=qb[quad][
                                    32 * bd : 32 * bd + KD,
                                    nt * NT : (nt + 1) * NT,
                                ],
                                start=True,
                                stop=True,
                                tile_position=(32 * bd, 0),
                            )
                        if carry and step <= 1:
                            f, cpm, cmc, cpair = carry.pop(0)
                            f(cpm, cmc, cpair)
                        if is_final and step >= 29:
                            lag = 2 if step == 29 else 1
                        else:
                            lag = PV_LAG
                        while len(pv_queue) > lag:
                            emit_pv(*pv_queue.pop(0))
                        if b == 0 and nt == 0:
                            deferred_b0(step)
                            if step == 0:
                                prologue_units.pop(0)()  # k1h nt0
                                units_fired[0] += 1
                                unit_next[0] = 1
                            elif step == 1:
                                prologue_units.pop(0)()  # q1h nt0
                                units_fired[0] += 1
                                unit_next[0] = 3
                        if nt == 0 and b + 1 < BPC and step == 4:
                            prefetch(b + 1)
                        stage_fired = False
                        if pending is not None and pending and step == pending[0][0]:
                            pending.pop(0)[1]()
                            stage_fired = True
                            if not pending:
                                pending = None
                        if (
                            not stage_fired
                            and prologue_units
                            and (step >= 4 or (b == 0 and nt == 0))
                            and step >= unit_next[0]
                        ):
                            prologue_units.pop(0)()
                            units_fired[0] += 1
                            sp = 2 if units_fired[0] <= 4 else 3
                            unit_next[0] = step + max(sp, unit_spacing)
                        p_sb = sb.tile([128, 2 * NT], BF16, tag="p", bufs=8, name="p")
                        nc.scalar.activation(
                            out=p_sb,
                            in_=cps,
                            func=mybir.ActivationFunctionType.Exp,
                            scale=0.25,
                        )
                        pm = sb.tile([128, 2 * NT], BF16, tag="pm", bufs=8, name="pm")
                        # stride-0 repeat of the adj row block across the two
                        # heads; adjT tiles are single-writer (one DMA each)
                        # so the raw AP's coarse dependency attribution is
                        # safe.
                        adj_src = adj_of(b, mc, nt)
                        adj_rep = bass.AP(
                            tensor=adj_src.tensor,
                            offset=adj_src.offset,
                            ap=[list(adj_src.ap[0]), [0, 2]]
                            + [list(a) for a in adj_src.ap[1:]],
                        )
                        nc.vector.tensor_mul(pm, p_sb, adj_rep)
                        if DEBUG and b == 0 and nt == 0 and step == 0:
                            nc.sync.dma_start(out=dbg_pm[:, :], in_=pm)
                        pv_queue.append((pm, mc, pair))
                        step += 1
                if is_final:
                    # drain with quad chains interleaved: pv_queue holds
                    # pairs 0 (step 30) and 1 (step 31) = quad 0.  Quad 1
                    # (pairs 2,3, PVs already emitted) chains on the idle
                    # ScalarE while quad 0's last PV waits on the last mask.
                    tail["hu"][1]()  # ScalarE, right after the last exp
                    emit_pv(*pv_queue.pop(0))  # PV pair 0
                    tail["ext"][1]()
                    emit_pv(*pv_queue.pop(0))  # PV pair 1 (waits last mask)
                    tail["recip"][1]()  # ScalarE: ln, exp(-x)
                    tail["hu"][0]()  # DVE, needs PV pair 1
                    tail["ext"][0]()
                    tail["recip"][0]()
                    tail["selmm"][1]()
                    tail["norm"][1]()
                    tail["selmm"][0]()
                    tail["norm"][0]()
                    tail["out"][0]()
                    tail["out"][1]()
                else:
                    # flush all but the last two PVs: their masks finish
                    # only after the segment's last exps, so emitting them
                    # here would block the PE ahead of the next segment's
                    # first compats.  They fire at the next steps 0/1.
                    while len(pv_queue) > 2:
                        emit_pv(*pv_queue.pop(0))
                    carry = [(emit_pv, pm_, mc_, pair_) for pm_, mc_, pair_ in pv_queue]
                    pv_queue = []
                    pending = epi
                unit_next[0] = 0
            del ios[b]
    return nc


def _split_multi_waits(nc):
    """walrus codegen in this container allows only one sync-wait per
    instruction; hoist extra waits onto preceding same-engine nops."""
    import copy
    import bass_rust

    tmpl_nc = bass.Bass()
    tmpls = {}
    for en in ["vector", "scalar", "tensor", "gpsimd", "sync"]:
        ins = getattr(tmpl_nc, en).nop().ins
        tmpls[str(ins.engine)] = ins

    uid = [0]
    for fn in nc.m.functions:
        for bb in fn.blocks:
            out = []
            for ins in bb.instructions:
                si = ins.sync_info
                waits = list(si.on_wait) if si is not None else []
                if len(waits) > 1:
                    for w in waits[:-1]:
                        nop = copy.deepcopy(tmpls[str(ins.engine)])
                        uid[0] += 1
                        nop.name = f"I-splitw-{uid[0]}"
                        nop.sync_info = bass_rust.SyncInfo(
                            on_wait=[w], on_update=[]
                        )
                        out.append(nop)
                    ins.sync_info = bass_rust.SyncInfo(
                        on_wait=[waits[-1]], on_update=list(si.on_update)
                    )
                out.append(ins)
            bb.instructions = out
    return nc


_cache = {}


def _get_nc():
    if "nc" not in _cache:
        _cache["nc"] = _split_multi_waits(build_kernel())
    return _cache["nc"]


def _prep_weights(W_query, W_key, W_val, W_out):
    bf = ml_dtypes.bfloat16
    wqb = np.zeros((2, E, 128), bf)
    wkb = np.zeros((2, E, 128), bf)
    wob = np.zeros((2, 128, E), bf)
    for h in range(H):
        q, c = h // 4, h % 4
        wqb[q, :, 32 * c : 32 * c + KD] = W_query[h].astype(bf)
        wkb[q, :, 32 * c : 32 * c + KD] = W_key[h].astype(bf)
        wob[q, 32 * c : 32 * c + KD, :] = W_out[h].astype(bf)
    wv = np.ascontiguousarray(
        np.asarray(W_val, np.float32).transpose(1, 0, 2).reshape(E, H * KD)
    ).astype(bf)
    ext = np.zeros((128, 128), bf)
    for c in range(4):
        ext[32 * c + KD, c] = 1.0  # sel4: extract den row of band c
    wpk = np.concatenate(
        [wkb[0], wqb[0], wkb[1], wqb[1], wv, wob[0], wob[1], ext], axis=1
    )
    sel = np.zeros((4, 128), ml_dtypes.bfloat16)
    for c in range(4):
        sel[c, 32 * c : 32 * c + KD + 1] = 1.0
    return np.ascontiguousarray(wpk), sel


def kernel(h, adj_c, W_query, W_key, W_val, W_out, trace=False):
    h = np.asarray(h, np.float32)
    adj = np.asarray(adj_c)
    bf = ml_dtypes.bfloat16
    hT = np.ascontiguousarray(h.transpose(0, 2, 1)).astype(bf)  # [B, E, N]
    # partition-contiguous layouts: one DMA descriptor per partition
    adjT = np.ascontiguousarray(
        adj.transpose(0, 2, 1).astype(bf).reshape(B, MC, 128, N).transpose(0, 2, 1, 3)
    )  # [B, 128, MC, N] bf16
    h_r = np.ascontiguousarray(
        h.reshape(B, MC, 128, E).transpose(0, 2, 1, 3)
    )  # [B, 128, MC, E]
    wpk, sel = _prep_weights(
        np.asarray(W_query, np.float32),
        np.asarray(W_key, np.float32),
        np.asarray(W_val, np.float32),
        np.asarray(W_out, np.float32),
    )

    nc = _get_nc()
    in_maps = []
    for c in range(CORES):
        s = slice(c * BPC, (c + 1) * BPC)
        in_maps.append(
            {
                "ht": np.ascontiguousarray(hT[s]),
                "hn": np.ascontiguousarray(h_r[s]),
                "adjt": np.ascontiguousarray(adjT[s]),
                "wpk": wpk,
                "sel": sel,
            }
        )
    res = run_bass_kernel_spmd(nc, in_maps, core_ids=list(range(CORES)), trace=trace)
    out = np.concatenate([r["out"] for r in res.results], axis=0)
    if trace:
        return out, res
    return out


# revision 59
# speedup vs baseline: 1.0051x; 1.0051x over previous
"""Multi-head graph attention (GAT-style) Trainium2 Bass kernel.

Full-input contract: kernel(**inputs) takes the complete arrays, shards
batch-wise across 8 NeuronCores (2 batches each), and gathers the output.

Math per batch b, head h (KD=16 head dim):
  Q = h @ Wq_h, K = h @ Wk_h, V = h @ Wv_h            [N, 16]
  compatT[m, n] = (K Q^T)[m, n]                        [N, N] (transposed)
  p = exp(0.25 * compatT) * adjT                       (mask after exp; exact:
      masked entries are exactly 0, matching softmax(-inf) * adj)
  headsT[v, n] = (V'.T @ p)  with V' = [V | 1 | 0pad]  -> row 16 = denominator
  out[n, :] = sum_h (headsT_h / den_h).T @ Wout_h + h[n, :]

Design: ACT's exp stream (128 calls x ~1.0us back-to-back) is the pacing
engine; every other engine hides under it.
 - Heads live in 32-partition bands (head h -> quad h//4, band h%4).
   Banded zero-padded projection weights put q/k of band b at SBUF
   partitions [32b, 32b+16), so the K=16 compat matmuls of a head PAIR
   run concurrently via PE row tiling (tile_position=(32b, 0)).
 - Each pair's two [128,512] compat tiles share one 2-bank PSUM buffer;
   one exp call covers both. Pair buffers rotate 3-deep (banks 0-5);
   PV accumulators for the two quads hold banks 6-7 per n-half.  The
   mask multiply runs one DVE op per pair (adjT operand repeated via a
   stride-0 AP); PV matmuls lag PV_LAG pairs behind compat in PE
   program order so the exp <- mask <- exp cross-engine cycle has
   several exp-periods of budget and DVE jitter never stalls ACT.
 - Mid-stream denominators: the 4 den rows (partition 32c+16) of each
   quad's PV accumulator are reshaped by gpsimd SBUF->SBUF DMAs into
   [128,16] so the exact DVE reciprocal uses all lanes (~250ns; the
   8-cyc/elem iterative divide makes a [4,512] recip 13x slower),
   scattered back to a [4,512] row tile, and band-broadcast by a
   selector matmul (sel.T @ rec4) on the mostly-idle PE.  Epilogue
   stages are software-pipelined INTO the next n-half's stream, spaced
   so only one stage borrows a pair PSUM buffer at a time (every borrow
   shrinks the compat rotation for one step, ~0.5us) and every consumer
   fires >=2 steps after its cross-engine producer (in-order queues: a
   too-early consumer head-of-line-blocks its whole engine).
 - Final-segment tail: pairs run quad-1-first so both quad chains
   overlap; quad 1's hu copy and both reciprocals ride the idle ScalarE
   as exp(-ln d) -- Ln/Exp share one ACT table set (natural_log_exp) so
   there is no table switch and no 8-cyc/elem DVE iterative divide.
 - A dummy 1-element exp at t=0 forces the ACT table load during the
   NEFF preamble.  First-batch DMAs are minimal and priority-ordered
   (wk0|wq0 block, then hT halves on two queues, then adj chunk 0); the
   bulk adj/h transfers are deferred into the stream so they don't
   steal HBM bandwidth from the critical path.  Quad 1's nt=0
   projections burst into steps 0-1 so the first compat+exp aren't
   queued behind them on the PE.
 - Every SBUF tile has exactly ONE writer DMA (wpk/hT/adj arrive in
   per-chunk tiles): a tile with two writer DMAs mis-attributes reads
   emitted close to the transfers and reads garbage.
 - All tiles live in two pools (one SBUF, one PSUM, per-tag buf counts)
   -- every pool costs a full cross-engine semaphore barrier mesh at
   teardown (~10us for an 11-pool layout).
 - adj / h ship in partition-contiguous host layouts (one descriptor
   per partition); adjT as bf16 (0/1 exact), h/q/k/v in bf16 (rel err
   ~3e-3 vs the 2e-2 gate).
"""

import os
import numpy as np
import ml_dtypes
from contextlib import ExitStack

import concourse.bass as bass
import concourse.mybir as mybir
import concourse.tile as tile
from concourse.bass_utils import run_bass_kernel_spmd

B, N, E, H, KD = 16, 1024, 128, 8, 16
CORES = 8
BPC = B // CORES  # batches per core
F32 = mybir.dt.float32
BF16 = mybir.dt.bfloat16
NT = 512  # n-half width (one PSUM bank of fp32 per [128, NT] tile)
MC = N // 128  # number of 128-row chunks of m
VP = 32  # padded per-head V columns (16 vals + 1 ones + 15 zeros)
NR = NT // 32  # free elems per lane in the [128, 2, NR] reciprocal
PV_LAG = 4


def build_kernel():
    nc = bass.Bass()
    hT_d = nc.dram_tensor("ht", [BPC, E, N], BF16, kind="ExternalInput")
    h_d = nc.dram_tensor("hn", [BPC, 128, MC, E], F32, kind="ExternalInput")
    adjt_d = nc.dram_tensor("adjt", [BPC, 128, MC, N], BF16, kind="ExternalInput")
    # packed weights: [wk0|wq0|wk1|wq1|wv|wo0|wo1|sel4pad] as [128, 8*128]
    wpk_d = nc.dram_tensor("wpk", [128, 8 * 128], BF16, kind="ExternalInput")
    sel_d = nc.dram_tensor("sel", [4, 128], BF16, kind="ExternalInput")
    out_d = nc.dram_tensor("out", [BPC, N, E], F32, kind="ExternalOutput")
    DEBUG = bool(int(os.environ.get("KDBG", "0")))
    if DEBUG:
        dbg_hu = nc.dram_tensor("dbg_hu", [2, 128, NT], F32, kind="ExternalOutput")
        dbg_d = nc.dram_tensor("dbg_d", [2, 128, NR], F32, kind="ExternalOutput")
        dbg_r = nc.dram_tensor("dbg_r", [2, 128, NR], BF16, kind="ExternalOutput")
        dbg_rec = nc.dram_tensor("dbg_rec", [2, 4, NT], BF16, kind="ExternalOutput")
        dbg_pm = nc.dram_tensor("dbg_pm", [128, 2 * NT], BF16, kind="ExternalOutput")

    with ExitStack() as ctx:
        tc = ctx.enter_context(tile.TileContext(nc))
        sb = ctx.enter_context(tc.tile_pool(name="sb", bufs=2))
        ps = ctx.enter_context(tc.tile_pool(name="ps", bufs=1, space="PSUM"))

        # packed weights land in two tiles so each has exactly ONE writer
        # DMA -- a tile with two writer DMAs mis-attributes reads emitted
        # close to the transfers (the b0 projection units) and reads
        # garbage.  wpk_a is the 64KB critical block (wk0|wq0).
        wpk_a = sb.tile([128, 256], BF16, tag="wpk_a", bufs=1)
        wpk_b = sb.tile([128, 6 * 128], BF16, tag="wpk_b", bufs=1)
        wk_q = [wpk_a[:, 0:128], wpk_b[:, 0:128]]
        wq_q = [wpk_a[:, 128:256], wpk_b[:, 128:256]]
        wv_sb = wpk_b[:, 256:384]
        wo_q = [wpk_b[:, 384:512], wpk_b[:, 512:640]]
        sel4_sb = wpk_b[:, 640:644]  # [128, 4]: ones at (32c+16, c)

        # band-broadcast selector: sel[c, 32c+j] = 1 (j < 17) so
        # (sel.T @ rec4) replicates rec4 row c into band c's rows on PE.
        sel_sb = sb.tile([4, 128], BF16, tag="sel", bufs=1)

        # dummy activation at t=0: walrus emits the ACT table load right
        # before it, so the ~2.7us load hides under the NEFF preamble.
        scratch = sb.tile([1, 8], F32, tag="scr", bufs=1)
        scratch_o = sb.tile([1, 8], BF16, tag="scro", bufs=1)
        nc.vector.memset(scratch, 0.0)
        nc.scalar.activation(
            out=scratch_o,
            in_=scratch,
            func=mybir.ActivationFunctionType.Exp,
            scale=0.25,
        )

        ios = {}

        def prefetch(b, first=False):
            if first:
                # priority-ordered critical set on two queues: wk0|wq0
                # block + hT halves + adj chunk 0.  The bulk transfers
                # (adj chunks 1-7, h) are deferred into the stream
                # (deferred_b0) so they don't steal HBM bandwidth from
                # the critical path.  Every tile has exactly one writer.
                hT_a = sb.tile([E, NT], BF16, tag="ht_a", bufs=1, name="hta")
                hT_b = sb.tile([E, NT], BF16, tag="ht_b", bufs=1, name="htb")
                adj_a = sb.tile([128, 1, N], BF16, tag="adj_a", bufs=1, name="aja")
                adj_b1 = sb.tile([128, 3, N], BF16, tag="adj_b1", bufs=1, name="ab1")
                adj_b2 = sb.tile(
                    [128, MC - 4, N], BF16, tag="adj_b2", bufs=1, name="ab2"
                )
                h_sb = sb.tile([128, MC, E], F32, tag="hn", name="hns")
                nc.gpsimd.dma_start(out=wpk_a, in_=wpk_d[:, 0:256])
                nc.sync.dma_start(out=hT_a, in_=hT_d[b, :, 0:NT])
                nc.gpsimd.dma_start(out=wpk_b, in_=wpk_d[:, 256:])
                nc.gpsimd.dma_start(out=sel_sb, in_=sel_d[:, :])
                nc.gpsimd.dma_start(out=hT_b, in_=hT_d[b, :, NT:N])
                nc.sync.dma_start(out=adj_a, in_=adjt_d[b, :, 0:1, :])
                ios[b] = {"ht": [hT_a, hT_b], "adj": [adj_a, adj_b1, adj_b2],
                          "h": h_sb}
            else:
                hT_sb = sb.tile([E, N], BF16, tag="ht", bufs=1, name="hts")
                adjT_sb = sb.tile([128, MC, N], BF16, tag="adj", bufs=1, name="adjs")
                h_sb = sb.tile([128, MC, E], F32, tag="hn", name="hns")
                nc.sync.dma_start(out=hT_sb, in_=hT_d[b, :, :])
                nc.sync.dma_start(out=adjT_sb, in_=adjt_d[b])
                nc.sync.dma_start(out=h_sb, in_=h_d[b])
                ios[b] = {"ht": [hT_sb], "adj": [adjT_sb], "h": h_sb}

        def ht_slice(b, c0, c1):
            parts = ios[b]["ht"]
            if len(parts) == 1:
                return parts[0][:, c0:c1]
            if c1 <= NT:
                return parts[0][:, c0:c1]
            return parts[1][:, c0 - NT : c1 - NT]

        def adj_of(b, mc, nt):
            parts = ios[b]["adj"]
            if len(parts) == 1:
                return parts[0][:, mc, nt * NT : (nt + 1) * NT]
            if mc == 0:
                return parts[0][:, 0, nt * NT : (nt + 1) * NT]
            if mc < 4:
                return parts[1][:, mc - 1, nt * NT : (nt + 1) * NT]
            return parts[2][:, mc - 4, nt * NT : (nt + 1) * NT]

        def deferred_b0(step):
            # bulk transfers for batch 0, after the critical DMAs
            if step == 0:
                nc.sync.dma_start(out=ios[0]["adj"][1], in_=adjt_d[0, :, 1:4, :])
            elif step == 1:
                nc.sync.dma_start(out=ios[0]["adj"][2], in_=adjt_d[0, :, 4:MC, :])
            elif step == 2:
                nc.sync.dma_start(out=ios[0]["h"], in_=h_d[0])

        bands = {}

        def make_prologue_units(b, split_first=False):
            """Projection + V-build for batch b as coarse closures.  Full
            units do both n-halves of a q/k row pair in one pair-buffer
            borrow; V-mega units build 4 m-chunks of V' at once.  For batch
            0 the first four units are nt=0-only so the first compat can
            issue as soon as the first hT half lands."""
            qb = [sb.tile([128, N], BF16, tag=f"q{q}", name=f"qb{q}") for q in range(2)]
            kb = [sb.tile([128, N], BF16, tag=f"k{q}", name=f"kb{q}") for q in range(2)]
            v_nat = [
                sb.tile([128, H, VP], BF16, tag=f"v{mc}", name=f"v{mc}")
                for mc in range(MC)
            ]
            bands[b] = (qb, kb, v_nat)

            def proj_half(w_sb, dst, nt):
                def run():
                    pp = ps.tile([128, NT], F32, tag="pair", bufs=3, name="pp")
                    nc.tensor.matmul(
                        out=pp,
                        lhsT=w_sb,
                        rhs=ht_slice(b, nt * NT, (nt + 1) * NT),
                        start=True,
                        stop=True,
                    )
                    nc.vector.tensor_copy(out=dst[:, nt * NT : (nt + 1) * NT], in_=pp)

                return run

            def proj_full(w_sb, dst):
                def run():
                    pp = ps.tile([128, 2, NT], F32, tag="pair", bufs=3, name="pp")
                    for nt in range(2):
                        nc.tensor.matmul(
                            out=pp[:, nt, :],
                            lhsT=w_sb,
                            rhs=ht_slice(b, nt * NT, (nt + 1) * NT),
                            start=True,
                            stop=True,
                        )
                    nc.vector.tensor_copy(
                        out=dst.rearrange("p (t n) -> p t n", t=2), in_=pp
                    )

                return run

            def v_unit(mc):
                def run():
                    vp = ps.tile([128, H * KD], F32, tag="pair", bufs=3, name="vp")
                    nc.tensor.matmul(
                        out=vp,
                        lhsT=ht_slice(b, mc * 128, (mc + 1) * 128),
                        rhs=wv_sb,
                        start=True,
                        stop=True,
                    )
                    vt = v_nat[mc]
                    nc.vector.tensor_copy(
                        out=vt[:, :, 0:KD],
                        in_=vp.rearrange("p (h k) -> p h k", k=KD),
                    )
                    nc.vector.memset(vt[:, :, KD : KD + 1], 1.0)
                    nc.vector.memset(vt[:, :, KD + 1 : VP], 0.0)

                return run

            if split_first:
                # pre-stream: only quad 0's nt=0 projections, so the first
                # compat+exp aren't queued behind quad 1's (whose wpk_b
                # weights land later).  k1h/v0/q1h burst into steps 0-1.
                first = [
                    proj_half(wk_q[0], kb[0], 0),
                    proj_half(wq_q[0], qb[0], 0),
                    v_unit(0),
                ]
                rest = [
                    proj_half(wk_q[1], kb[1], 0),
                    proj_half(wq_q[1], qb[1], 0),
                    v_unit(1),
                    v_unit(2),
                    v_unit(3),
                    proj_half(wk_q[0], kb[0], 1),
                    proj_half(wk_q[1], kb[1], 1),
                    v_unit(4),
                    v_unit(5),
                    v_unit(6),
                    v_unit(7),
                    proj_half(wq_q[0], qb[0], 1),
                    proj_half(wq_q[1], qb[1], 1),
                ]
                return first, rest
            return None, [
                proj_half(wk_q[0], kb[0], 0),
                proj_half(wq_q[0], qb[0], 0),
                proj_half(wk_q[1], kb[1], 0),
                proj_half(wq_q[1], qb[1], 0),
                v_unit(0),
                v_unit(1),
                v_unit(2),
                v_unit(3),
                proj_half(wk_q[0], kb[0], 1),
                proj_half(wk_q[1], kb[1], 1),
                v_unit(4),
                v_unit(5),
                v_unit(6),
                v_unit(7),
                proj_half(wq_q[0], qb[0], 1),
                proj_half(wq_q[1], qb[1], 1),
            ]

        def make_epilogue(b, nt, hp):
            """Normalize + W_out + residual for (b, nt) as staged closures.
            Stages are spaced so at most one borrows a pair PSUM buffer at
            a time, and every consumer fires >=2 steps after its producer."""
            h_sb = ios[b]["h"]
            hus, hn2, oms = {}, {}, {}
            st = {}

            dbg = DEBUG and b == 0 and nt == 0

            def hu_q(q):
                def run():
                    hu = sb.tile([128, NT], F32, tag=f"hu{q}", name=f"hu{q}")
                    nc.vector.tensor_copy(out=hu, in_=hp[q])
                    hus[q] = hu
                    if dbg:
                        nc.sync.dma_start(out=dbg_hu[q, :, :], in_=hu)
                return run

            def gather_q(q):
                def run():
                    d128 = sb.tile([128, NR], F32, tag=f"d128{q}", name=f"d1{q}")
                    for c in range(4):
                        srcp = hus[q][32 * c + KD : 32 * c + KD + 1, :]
                        src_r = bass.AP(
                            tensor=srcp.tensor,
                            offset=srcp.offset,
                            ap=[list(srcp.ap[0]), [NR, 32], [1, NR]],
                        )
                        nc.gpsimd.dma_start(
                            out=d128[32 * c : 32 * c + 32, :], in_=src_r
                        )
                    st[f"d128{q}"] = d128
                    if dbg:
                        nc.sync.dma_start(out=dbg_d[q, :, :], in_=d128)
                return run

            def recip_q(q):
                def run():
                    r128b = sb.tile([128, NR], BF16, tag=f"r128b{q}", name=f"rb{q}")
                    with nc.allow_low_precision(
                        reason="denominator recip to bf16 is within gate"
                    ):
                        nc.vector.reciprocal(out=r128b, in_=st[f"d128{q}"])
                    st[f"r128b{q}"] = r128b
                    if dbg:
                        nc.sync.dma_start(out=dbg_r[q, :, :], in_=r128b)
                return run

            def scatter_both():
                for q in range(2):
                    rec4 = sb.tile([4, NT], BF16, tag=f"rec4{q}", name=f"rc{q}")
                    dst = rec4[:, :]
                    dst_r = bass.AP(
                        tensor=dst.tensor,
                        offset=dst.offset,
                        ap=[list(dst.ap[0]), [NR, 32], [1, NR]],
                    )
                    nc.gpsimd.dma_start(out=dst_r, in_=st[f"r128b{q}"])
                    st[f"rec4{q}"] = rec4
                    if dbg:
                        nc.sync.dma_start(out=dbg_rec[q, :, :], in_=rec4)

            def selmm_both():
                bc_ps = ps.tile([128, 2, NT], F32, tag="pair", bufs=3, name="bc")
                for q in range(2):
                    nc.tensor.matmul(
                        out=bc_ps[:, q, :],
                        lhsT=sel_sb,
                        rhs=st[f"rec4{q}"],
                        start=True,
                        stop=True,
                    )
                st["bc"] = bc_ps

            def norm_q(q):
                def run():
                    hn = sb.tile([128, NT], BF16, tag=f"hn{q}", name=f"hn{q}")
                    nc.vector.tensor_mul(hn, hus[q], st["bc"][:, q, :])
                    hn2[q] = hn
                return run

            def out_mm_all():
                o_ps = ps.tile([128, 4, E], F32, tag="pair", bufs=3, name="op")
                for cl in range(4):
                    for q in range(2):
                        nc.tensor.matmul(
                            out=o_ps[:, cl, :],
                            lhsT=hn2[q][:, cl * 128 : (cl + 1) * 128],
                            rhs=wo_q[q],
                            start=(q == 0),
                            stop=(q == 1),
                        )
                st["o_ps"] = o_ps

            def out_fin_all():
                cc = nt * (NT // 128)
                ob = sb.tile([128, 4, E], F32, tag="ob", name="ob")
                nc.vector.tensor_add(ob, st["o_ps"], h_sb[:, cc : cc + 4, :])
                nc.sync.dma_start(
                    out=out_d[b, cc * 128 : (cc + 4) * 128, :].rearrange(
                        "(c p) e -> p c e", p=128
                    ),
                    in_=ob,
                )

            # spacing: at most one pair-PSUM borrow live at a time (the
            # merged bc until both norms, the merged o_ps until out_fin),
            # and every cross-engine consumer fires >=2 steps after its
            # producer so in-order queues never head-of-line block.
            stages = [
                (1, hu_q(0)),
                (3, hu_q(1)),
                (5, gather_q(0)),
                (7, gather_q(1)),
                (9, recip_q(0)),
                (10, recip_q(1)),
                (11, scatter_both),
                (13, selmm_both),
                (15, norm_q(0)),
                (17, norm_q(1)),
                (19, out_mm_all),
                (22, out_fin_all),
            ]
            return stages

        def make_tail_epilogue(b, nt, hp):
            """Latency-optimized final chain.  Quads finish in order 1 then
            0 (the final segment streams pairs 2,3,0,1).  Quad 1's hu copy
            rides the ScalarE (idle once exps end); reciprocals run as
            exp(-ln d) on ScalarE -- Ln and Exp share one ACT table set
            (natural_log_exp) so there is no table switch and the
            8-cyc/elem DVE iterative divide is avoided."""
            h_sb = ios[b]["h"]
            hus, den_ps, rec4s, rec_ps, hn2 = {}, {}, {}, {}, {}

            def hu_q(q, eng):
                def run():
                    hu = sb.tile([128, NT], BF16, tag=f"hut{q}", name=f"hut{q}")
                    if eng == "scalar":
                        nc.scalar.copy(out=hu, in_=hp[q])
                    else:
                        nc.vector.tensor_copy(out=hu, in_=hp[q])
                    hus[q] = hu
                return run

            def ext_q(q):
                def run():
                    d4 = ps.tile([4, NT], F32, tag="pair", bufs=3, name="d4")
                    nc.tensor.matmul(
                        out=d4, lhsT=sel4_sb, rhs=hus[q], start=True, stop=True
                    )
                    den_ps[q] = d4
                return run

            def recip_q(q):
                def run():
                    lnd = sb.tile([4, NT], F32, tag=f"lnd{q}", name=f"ln{q}")
                    nc.scalar.activation(
                        out=lnd,
                        in_=den_ps[q],
                        func=mybir.ActivationFunctionType.Ln,
                    )
                    rec4 = sb.tile([4, NT], BF16, tag=f"rect{q}", name=f"rt{q}")
                    with nc.allow_low_precision(
                        reason="denominator recip to bf16 is within gate"
                    ):
                        nc.scalar.activation(
                            out=rec4,
                            in_=lnd,
                            func=mybir.ActivationFunctionType.Exp,
                            scale=-1.0,
                        )
                    rec4s[q] = rec4
                return run

            def selmm_q(q):
                def run():
                    bc_ps = ps.tile([128, NT], F32, tag="pair", bufs=3, name="bc")
                    nc.tensor.matmul(
                        out=bc_ps, lhsT=sel_sb, rhs=rec4s[q], start=True, stop=True
                    )
                    rec_ps[q] = bc_ps
                return run

            def norm_q(q):
                def run():
                    hn = sb.tile([128, NT], BF16, tag=f"hn{q}", name=f"hn{q}")
                    nc.vector.tensor_mul(hn, hus[q], rec_ps[q])
                    hn2[q] = hn
                return run

            def out_cp(cp):
                def run():
                    cc = nt * (NT // 128) + cp * 2
                    o_ps = ps.tile([128, 2, E], F32, tag="pair", bufs=3, name="op")
                    for cl in range(2):
                        for q in range(2):
                            nc.tensor.matmul(
                                out=o_ps[:, cl, :],
                                lhsT=hn2[q][
                                    :,
                                    (cp * 2 + cl) * 128 : (cp * 2 + cl + 1) * 128,
                                ],
                                rhs=wo_q[q],
                                start=(q == 0),
                                stop=(q == 1),
                            )
                    ob = sb.tile([128, 2, E], F32, tag="ob", name="ob")
                    nc.vector.tensor_add(ob, o_ps, h_sb[:, cc : cc + 2, :])
                    nc.sync.dma_start(
                        out=out_d[b, cc * 128 : (cc + 2) * 128, :].rearrange(
                            "(c p) e -> p c e", p=128
                        ),
                        in_=ob,
                    )
                return run

            return {
                "hu": {q: hu_q(q, "scalar" if q == 1 else "vector") for q in range(2)},
                "ext": {q: ext_q(q) for q in range(2)},
                "recip": {q: recip_q(q) for q in range(2)},
                "selmm": {q: selmm_q(q) for q in range(2)},
                "norm": {q: norm_q(q) for q in range(2)},
                "out": [out_cp(0), out_cp(1)],
            }

        # ---- main stream over (batch, n-half) segments ----
        prefetch(0, first=True)
        b0_first, b0_rest = make_prologue_units(0, split_first=True)
        for u in b0_first:
            u()
        pending = None
        prologue_units = b0_rest
        unit_next = [0]
        units_fired = [0]
        unit_spacing = 2
        tail = None
        carry = []  # last 2 PVs of the previous segment (masks not yet done)
        for b in range(BPC):
            qb, kb, v_nat = bands.pop(b)
            for nt in range(N // NT):
                is_final = b == BPC - 1 and nt == N // NT - 1
                if nt == 1 and b + 1 < BPC:
                    prologue_units = (
                        prologue_units + make_prologue_units(b + 1)[1]
                    )
                    unit_spacing = 3
                hp = [
                    ps.tile([128, NT], F32, tag=f"hp{q}", bufs=1, name=f"hp{q}")
                    for q in range(2)
                ]
                if is_final:
                    tail = make_tail_epilogue(b, nt, hp)
                    pair_order = [2, 3, 0, 1]  # quad 1 finishes first
                else:
                    epi = make_epilogue(b, nt, hp)
                    pair_order = [0, 1, 2, 3]
                pv_queue = []

                def emit_pv(pm, mc, pair, hp=hp, v_nat=v_nat):
                    # hp/v_nat bound by VALUE: carried entries fire in the
                    # next segment, after the loop variables move on.
                    for j in range(2):
                        hh = pair * 2 + j
                        c = hh % 4
                        nc.tensor.matmul(
                            out=hp[hh // 4][32 * c : 32 * c + VP, :],
                            lhsT=v_nat[mc][:, hh, :],
                            rhs=pm[:, j * NT : (j + 1) * NT],
                            start=(mc == 0),
                            stop=(mc == MC - 1),
                            tile_position=(0, 32 * c),
                        )

                step = 0
                for mc in range(MC):
                    for pair in pair_order:
                        quad, b0 = pair // 2, (pair * 2) % 4
                        cps = ps.tile(
                            [128, 2 * NT], F32, tag="pair", bufs=3, name="cp"
                        )
                        for j in range(2):
                            bd = b0 + j
                            nc.tensor.matmul(
                                out=cps[:, j * NT : (j + 1) * NT],
                                lhsT=kb[quad][
                                    32 * bd : 32 * bd + KD,
                                    mc * 128 : (mc + 1) * 128,
                                ],
                                rhs=qb[quad][
                                    32 * bd : 32 * bd + KD,
                                    nt * NT : (nt + 1) * NT,
                                ],
                                start=True,
                                stop=True,
                                tile_position=(32 * bd, 0),
                            )
                        if carry and step <= 1:
                            f, cpm, cmc, cpair = carry.pop(0)
                            f(cpm, cmc, cpair)
                        if is_final and step >= 29:
                            lag = 2 if step == 29 else 1
                        else:
                            lag = PV_LAG
                        while len(pv_queue) > lag:
                            emit_pv(*pv_queue.pop(0))
                        if b == 0 and nt == 0:
                            deferred_b0(step)
                            if step == 0:
                                prologue_units.pop(0)()  # k1h nt0
                                units_fired[0] += 1
                                unit_next[0] = 1
                            elif step == 1:
                                prologue_units.pop(0)()  # q1h nt0
                                units_fired[0] += 1
                                unit_next[0] = 3
                        if nt == 0 and b + 1 < BPC and step == 4:
                            prefetch(b + 1)
                        stage_fired = False
                        if pending is not None and pending and step == pending[0][0]:
                            pending.pop(0)[1]()
                            stage_fired = True
                            if not pending:
                                pending = None
                        if (
                            not stage_fired
                            and prologue_units
                            and (step >= 4 or (b == 0 and nt == 0))
                            and step >= unit_next[0]
                        ):
                            prologue_units.pop(0)()
                            units_fired[0] += 1
                            # tighten near segment end so the last units'
                            # pool-slot displacement doesn't land on the
                            # next segment's first compats
                            if units_fired[0] <= 4:
                                sp = 2
                            elif len(prologue_units) <= 2 and step >= 20:
                                sp = 2
                            else:
                                sp = 3
                            unit_next[0] = step + max(sp, unit_spacing)
                        p_sb = sb.tile([128, 2 * NT], BF16, tag="p", bufs=8, name="p")
                        nc.scalar.activation(
                            out=p_sb,
                            in_=cps,
                            func=mybir.ActivationFunctionType.Exp,
                            scale=0.25,
                        )
                        pm = sb.tile([128, 2 * NT], BF16, tag="pm", bufs=8, name="pm")
                        # stride-0 repeat of the adj row block across the two
                        # heads; adjT tiles are single-writer (one DMA each)
                        # so the raw AP's coarse dependency attribution is
                        # safe.
                        adj_src = adj_of(b, mc, nt)
                        adj_rep = bass.AP(
                            tensor=adj_src.tensor,
                            offset=adj_src.offset,
                            ap=[list(adj_src.ap[0]), [0, 2]]
                            + [list(a) for a in adj_src.ap[1:]],
                        )
                        nc.vector.tensor_mul(pm, p_sb, adj_rep)
                        if DEBUG and b == 0 and nt == 0 and step == 0:
                            nc.sync.dma_start(out=dbg_pm[:, :], in_=pm)
                        pv_queue.append((pm, mc, pair))
                        step += 1
                if is_final:
                    # drain with quad chains interleaved: pv_queue holds
                    # pairs 0 (step 30) and 1 (step 31) = quad 0.  Quad 1
                    # (pairs 2,3, PVs already emitted) chains on the idle
                    # ScalarE while quad 0's last PV waits on the last mask.
                    tail["hu"][1]()  # ScalarE, right after the last exp
                    emit_pv(*pv_queue.pop(0))  # PV pair 0
                    tail["ext"][1]()
                    emit_pv(*pv_queue.pop(0))  # PV pair 1 (waits last mask)
                    tail["recip"][1]()  # ScalarE: ln, exp(-x)
                    tail["hu"][0]()  # DVE, needs PV pair 1
                    tail["ext"][0]()
                    tail["recip"][0]()
                    tail["selmm"][1]()
                    tail["norm"][1]()
                    tail["selmm"][0]()
                    tail["norm"][0]()
                    tail["out"][0]()
                    tail["out"][1]()
                else:
                    # flush all but the last two PVs: their masks finish
                    # only after the segment's last exps, so emitting them
                    # here would block the PE ahead of the next segment's
                    # first compats.  They fire at the next steps 0/1.
                    while len(pv_queue) > 2:
                        emit_pv(*pv_queue.pop(0))
                    carry = [(emit_pv, pm_, mc_, pair_) for pm_, mc_, pair_ in pv_queue]
                    pv_queue = []
                    pending = epi
                unit_next[0] = 0
            del ios[b]
    return nc


def _split_multi_waits(nc):
    """walrus codegen in this container allows only one sync-wait per
    instruction; hoist extra waits onto preceding same-engine nops."""
    import copy
    import bass_rust

    tmpl_nc = bass.Bass()
    tmpls = {}
    for en in ["vector", "scalar", "tensor", "gpsimd", "sync"]:
        ins = getattr(tmpl_nc, en).nop().ins
        tmpls[str(ins.engine)] = ins

    uid = [0]
    for fn in nc.m.functions:
        for bb in fn.blocks:
            out = []
            for ins in bb.instructions:
                si = ins.sync_info
                waits = list(si.on_wait) if si is not None else []
                if len(waits) > 1:
                    for w in waits[:-1]:
                        nop = copy.deepcopy(tmpls[str(ins.engine)])
                        uid[0] += 1
                        nop.name = f"I-splitw-{uid[0]}"
                        nop.sync_info = bass_rust.SyncInfo(
                            on_wait=[w], on_update=[]
                        )
                        out.append(nop)
                    ins.sync_info = bass_rust.SyncInfo(
                        on_wait=[waits[-1]], on_update=list(si.on_update)
                    )
                out.append(ins)
            bb.instructions = out
    return nc


_cache = {}


def _get_nc():
    if "nc" not in _cache:
        _cache["nc"] = _split_multi_waits(build_kernel())
    return _cache["nc"]


def _prep_weights(W_query, W_key, W_val, W_out):
    bf = ml_dtypes.bfloat16
    wqb = np.zeros((2, E, 128), bf)
    wkb = np.zeros((2, E, 128), bf)
    wob = np.zeros((2, 128, E), bf)
    for h in range(H):
        q, c = h // 4, h % 4
        wqb[q, :, 32 * c : 32 * c + KD] = W_query[h].astype(bf)
        wkb[q, :, 32 * c : 32 * c + KD] = W_key[h].astype(bf)
        wob[q, 32 * c : 32 * c + KD, :] = W_out[h].astype(bf)
    wv = np.ascontiguousarray(
        np.asarray(W_val, np.float32).transpose(1, 0, 2).reshape(E, H * KD)
    ).astype(bf)
    ext = np.zeros((128, 128), bf)
    for c in range(4):
        ext[32 * c + KD, c] = 1.0  # sel4: extract den row of band c
    wpk = np.concatenate(
        [wkb[0], wqb[0], wkb[1], wqb[1], wv, wob[0], wob[1], ext], axis=1
    )
    sel = np.zeros((4, 128), ml_dtypes.bfloat16)
    for c in range(4):
        sel[c, 32 * c : 32 * c + KD + 1] = 1.0
    return np.ascontiguousarray(wpk), sel


def kernel(h, adj_c, W_query, W_key, W_val, W_out, trace=False):
    h = np.asarray(h, np.float32)
    adj = np.asarray(adj_c)
    bf = ml_dtypes.bfloat16
    hT = np.ascontiguousarray(h.transpose(0, 2, 1)).astype(bf)  # [B, E, N]
    # partition-contiguous layouts: one DMA descriptor per partition
    adjT = np.ascontiguousarray(
        adj.transpose(0, 2, 1).astype(bf).reshape(B, MC, 128, N).transpose(0, 2, 1, 3)
    )  # [B, 128, MC, N] bf16
    h_r = np.ascontiguousarray(
        h.reshape(B, MC, 128, E).transpose(0, 2, 1, 3)
    )  # [B, 128, MC, E]
    wpk, sel = _prep_weights(
        np.asarray(W_query, np.float32),
        np.asarray(W_key, np.float32),
        np.asarray(W_val, np.float32),
        np.asarray(W_out, np.float32),
    )

    nc = _get_nc()
    in_maps = []
    for c in range(CORES):
        s = slice(c * BPC, (c + 1) * BPC)
        in_maps.append(
            {
                "ht": np.ascontiguousarray(hT[s]),
                "hn": np.ascontiguousarray(h_r[s]),
                "adjt": np.ascontiguousarray(adjT[s]),
                "wpk": wpk,
                "sel": sel,
            }
        )
    res = run_bass_kernel_spmd(nc, in_maps, core_ids=list(range(CORES)), trace=trace)
    out = np.concatenate([r["out"] for r in res.results], axis=0)
    if trace:
        return out, res
    return out


# revision 60
# speedup vs baseline: 1.0130x; 1.0079x over previous
"""Multi-head graph attention (GAT-style) Trainium2 Bass kernel.

Full-input contract: kernel(**inputs) takes the complete arrays, shards
batch-wise across 8 NeuronCores (2 batches each), and gathers the output.

Math per batch b, head h (KD=16 head dim):
  Q = h @ Wq_h, K = h @ Wk_h, V = h @ Wv_h            [N, 16]
  compatT[m, n] = (K Q^T)[m, n]                        [N, N] (transposed)
  p = exp(0.25 * compatT) * adjT                       (mask after exp; exact:
      masked entries are exactly 0, matching softmax(-inf) * adj)
  headsT[v, n] = (V'.T @ p)  with V' = [V | 1 | 0pad]  -> row 16 = denominator
  out[n, :] = sum_h (headsT_h / den_h).T @ Wout_h + h[n, :]

Design: ACT's exp stream (128 calls x ~1.0us back-to-back) is the pacing
engine; every other engine hides under it.
 - Heads live in 32-partition bands (head h -> quad h//4, band h%4).
   Banded zero-padded projection weights put q/k of band b at SBUF
   partitions [32b, 32b+16), so the K=16 compat matmuls of a head PAIR
   run concurrently via PE row tiling (tile_position=(32b, 0)).
 - Each pair's two [128,512] compat tiles share one 2-bank PSUM buffer;
   one exp call covers both. Pair buffers rotate 3-deep (banks 0-5);
   PV accumulators for the two quads hold banks 6-7 per n-half.  The
   mask multiply runs one DVE op per pair (adjT operand repeated via a
   stride-0 AP); PV matmuls lag PV_LAG pairs behind compat in PE
   program order so the exp <- mask <- exp cross-engine cycle has
   several exp-periods of budget and DVE jitter never stalls ACT.
 - Mid-stream denominators: the 4 den rows (partition 32c+16) of each
   quad's PV accumulator are reshaped by gpsimd SBUF->SBUF DMAs into
   [128,16] so the exact DVE reciprocal uses all lanes (~250ns; the
   8-cyc/elem iterative divide makes a [4,512] recip 13x slower),
   scattered back to a [4,512] row tile, and band-broadcast by a
   selector matmul (sel.T @ rec4) on the mostly-idle PE.  Epilogue
   stages are software-pipelined INTO the next n-half's stream, spaced
   so only one stage borrows a pair PSUM buffer at a time (every borrow
   shrinks the compat rotation for one step, ~0.5us) and every consumer
   fires >=2 steps after its cross-engine producer (in-order queues: a
   too-early consumer head-of-line-blocks its whole engine).
 - Final-segment tail: pairs run quad-1-first so both quad chains
   overlap; quad 1's hu copy and both reciprocals ride the idle ScalarE
   as exp(-ln d) -- Ln/Exp share one ACT table set (natural_log_exp) so
   there is no table switch and no 8-cyc/elem DVE iterative divide.
 - A dummy 1-element exp at t=0 forces the ACT table load during the
   NEFF preamble.  First-batch DMAs are minimal and priority-ordered
   (wk0|wq0 block, then hT halves on two queues, then adj chunk 0); the
   bulk adj/h transfers are deferred into the stream so they don't
   steal HBM bandwidth from the critical path.  Quad 1's nt=0
   projections burst into steps 0-1 so the first compat+exp aren't
   queued behind them on the PE.
 - Every SBUF tile has exactly ONE writer DMA (wpk/hT/adj arrive in
   per-chunk tiles): a tile with two writer DMAs mis-attributes reads
   emitted close to the transfers and reads garbage.
 - All tiles live in two pools (one SBUF, one PSUM, per-tag buf counts)
   -- every pool costs a full cross-engine semaphore barrier mesh at
   teardown (~10us for an 11-pool layout).
 - adj / h ship in partition-contiguous host layouts (one descriptor
   per partition); adjT as bf16 (0/1 exact), h/q/k/v in bf16 (rel err
   ~3e-3 vs the 2e-2 gate).
"""

import os
import numpy as np
import ml_dtypes
from contextlib import ExitStack

import concourse.bass as bass
import concourse.mybir as mybir
import concourse.tile as tile
from concourse.bass_utils import run_bass_kernel_spmd

B, N, E, H, KD = 16, 1024, 128, 8, 16
CORES = 8
BPC = B // CORES  # batches per core
F32 = mybir.dt.float32
BF16 = mybir.dt.bfloat16
NT = 512  # n-half width (one PSUM bank of fp32 per [128, NT] tile)
MC = N // 128  # number of 128-row chunks of m
VP = 32  # padded per-head V columns (16 vals + 1 ones + 15 zeros)
NR = NT // 32  # free elems per lane in the [128, 2, NR] reciprocal
PV_LAG = 4


def build_kernel():
    nc = bass.Bass()
    hT_d = nc.dram_tensor("ht", [BPC, E, N], BF16, kind="ExternalInput")
    h_d = nc.dram_tensor("hn", [BPC, 128, MC, E], F32, kind="ExternalInput")
    adjt_d = nc.dram_tensor("adjt", [BPC, 128, MC, N], BF16, kind="ExternalInput")
    # packed weights: [wk0|wq0|wk1|wq1|wv|wo0|wo1|sel4pad] as [128, 8*128]
    wpk_d = nc.dram_tensor("wpk", [128, 8 * 128], BF16, kind="ExternalInput")
    sel_d = nc.dram_tensor("sel", [4, 128], BF16, kind="ExternalInput")
    out_d = nc.dram_tensor("out", [BPC, N, E], F32, kind="ExternalOutput")
    DEBUG = bool(int(os.environ.get("KDBG", "0")))
    if DEBUG:
        dbg_hu = nc.dram_tensor("dbg_hu", [2, 128, NT], F32, kind="ExternalOutput")
        dbg_d = nc.dram_tensor("dbg_d", [2, 128, NR], F32, kind="ExternalOutput")
        dbg_r = nc.dram_tensor("dbg_r", [2, 128, NR], BF16, kind="ExternalOutput")
        dbg_rec = nc.dram_tensor("dbg_rec", [2, 4, NT], BF16, kind="ExternalOutput")
        dbg_pm = nc.dram_tensor("dbg_pm", [128, 2 * NT], BF16, kind="ExternalOutput")

    with ExitStack() as ctx:
        tc = ctx.enter_context(tile.TileContext(nc))
        sb = ctx.enter_context(tc.tile_pool(name="sb", bufs=2))
        ps = ctx.enter_context(tc.tile_pool(name="ps", bufs=1, space="PSUM"))

        # packed weights land in two tiles so each has exactly ONE writer
        # DMA -- a tile with two writer DMAs mis-attributes reads emitted
        # close to the transfers (the b0 projection units) and reads
        # garbage.  wpk_a is the 64KB critical block (wk0|wq0).
        wpk_a = sb.tile([128, 256], BF16, tag="wpk_a", bufs=1)
        wpk_b = sb.tile([128, 6 * 128], BF16, tag="wpk_b", bufs=1)
        wk_q = [wpk_a[:, 0:128], wpk_b[:, 0:128]]
        wq_q = [wpk_a[:, 128:256], wpk_b[:, 128:256]]
        wv_sb = wpk_b[:, 256:384]
        wo_q = [wpk_b[:, 384:512], wpk_b[:, 512:640]]
        sel4_sb = wpk_b[:, 640:644]  # [128, 4]: ones at (32c+16, c)

        # band-broadcast selector: sel[c, 32c+j] = 1 (j < 17) so
        # (sel.T @ rec4) replicates rec4 row c into band c's rows on PE.
        sel_sb = sb.tile([4, 128], BF16, tag="sel", bufs=1)

        # dummy activation at t=0: walrus emits the ACT table load right
        # before it, so the ~2.7us load hides under the NEFF preamble.
        scratch = sb.tile([1, 8], F32, tag="scr", bufs=1)
        scratch_o = sb.tile([1, 8], BF16, tag="scro", bufs=1)
        nc.vector.memset(scratch, 0.0)
        nc.scalar.activation(
            out=scratch_o,
            in_=scratch,
            func=mybir.ActivationFunctionType.Exp,
            scale=0.25,
        )

        ios = {}

        def prefetch(b, first=False):
            if first:
                # priority-ordered critical set on two queues: wk0|wq0
                # block + hT halves + adj chunk 0.  The bulk transfers
                # (adj chunks 1-7, h) are deferred into the stream
                # (deferred_b0) so they don't steal HBM bandwidth from
                # the critical path.  Every tile has exactly one writer.
                hT_a = sb.tile([E, NT], BF16, tag="ht_a", bufs=1, name="hta")
                hT_b = sb.tile([E, NT], BF16, tag="ht_b", bufs=1, name="htb")
                adj_a = sb.tile([128, 1, N], BF16, tag="adj_a", bufs=1, name="aja")
                adj_b1 = sb.tile([128, 3, N], BF16, tag="adj_b1", bufs=1, name="ab1")
                adj_b2 = sb.tile(
                    [128, MC - 4, N], BF16, tag="adj_b2", bufs=1, name="ab2"
                )
                h_sb = sb.tile([128, MC, E], F32, tag="hn", name="hns")
                nc.gpsimd.dma_start(out=wpk_a, in_=wpk_d[:, 0:256])
                nc.sync.dma_start(out=hT_a, in_=hT_d[b, :, 0:NT])
                nc.gpsimd.dma_start(out=wpk_b, in_=wpk_d[:, 256:])
                nc.gpsimd.dma_start(out=sel_sb, in_=sel_d[:, :])
                nc.gpsimd.dma_start(out=hT_b, in_=hT_d[b, :, NT:N])
                nc.sync.dma_start(out=adj_a, in_=adjt_d[b, :, 0:1, :])
                ios[b] = {"ht": [hT_a, hT_b], "adj": [adj_a, adj_b1, adj_b2],
                          "h": h_sb}
            else:
                hT_sb = sb.tile([E, N], BF16, tag="ht", bufs=1, name="hts")
                adjT_sb = sb.tile([128, MC, N], BF16, tag="adj", bufs=1, name="adjs")
                h_sb = sb.tile([128, MC, E], F32, tag="hn", name="hns")
                nc.sync.dma_start(out=hT_sb, in_=hT_d[b, :, :])
                nc.sync.dma_start(out=adjT_sb, in_=adjt_d[b])
                nc.sync.dma_start(out=h_sb, in_=h_d[b])
                ios[b] = {"ht": [hT_sb], "adj": [adjT_sb], "h": h_sb}

        def ht_slice(b, c0, c1):
            parts = ios[b]["ht"]
            if len(parts) == 1:
                return parts[0][:, c0:c1]
            if c1 <= NT:
                return parts[0][:, c0:c1]
            return parts[1][:, c0 - NT : c1 - NT]

        def adj_of(b, mc, nt):
            parts = ios[b]["adj"]
            if len(parts) == 1:
                return parts[0][:, mc, nt * NT : (nt + 1) * NT]
            if mc == 0:
                return parts[0][:, 0, nt * NT : (nt + 1) * NT]
            if mc < 4:
                return parts[1][:, mc - 1, nt * NT : (nt + 1) * NT]
            return parts[2][:, mc - 4, nt * NT : (nt + 1) * NT]

        def deferred_b0(step):
            # bulk transfers for batch 0, after the critical DMAs
            if step == 0:
                nc.sync.dma_start(out=ios[0]["adj"][1], in_=adjt_d[0, :, 1:4, :])
            elif step == 1:
                nc.sync.dma_start(out=ios[0]["adj"][2], in_=adjt_d[0, :, 4:MC, :])
            elif step == 2:
                nc.sync.dma_start(out=ios[0]["h"], in_=h_d[0])

        bands = {}

        def make_prologue_units(b, split_first=False):
            """Projection + V-build for batch b as coarse closures.  Full
            units do both n-halves of a q/k row pair in one pair-buffer
            borrow; V-mega units build 4 m-chunks of V' at once.  For batch
            0 the first four units are nt=0-only so the first compat can
            issue as soon as the first hT half lands."""
            qb = [sb.tile([128, N], BF16, tag=f"q{q}", name=f"qb{q}") for q in range(2)]
            kb = [sb.tile([128, N], BF16, tag=f"k{q}", name=f"kb{q}") for q in range(2)]
            v_nat = [
                sb.tile([128, H, VP], BF16, tag=f"v{mc}", name=f"v{mc}")
                for mc in range(MC)
            ]
            bands[b] = (qb, kb, v_nat)

            def proj_half(w_sb, dst, nt):
                def run(ptag="pair"):
                    pp = ps.tile(
                        [128, NT], F32, tag=ptag,
                        bufs=3 if ptag == "pair" else 1, name="pp"
                    )
                    nc.tensor.matmul(
                        out=pp,
                        lhsT=w_sb,
                        rhs=ht_slice(b, nt * NT, (nt + 1) * NT),
                        start=True,
                        stop=True,
                    )
                    nc.vector.tensor_copy(out=dst[:, nt * NT : (nt + 1) * NT], in_=pp)

                return run

            def proj_full(w_sb, dst):
                def run():
                    pp = ps.tile([128, 2, NT], F32, tag="pair", bufs=3, name="pp")
                    for nt in range(2):
                        nc.tensor.matmul(
                            out=pp[:, nt, :],
                            lhsT=w_sb,
                            rhs=ht_slice(b, nt * NT, (nt + 1) * NT),
                            start=True,
                            stop=True,
                        )
                    nc.vector.tensor_copy(
                        out=dst.rearrange("p (t n) -> p t n", t=2), in_=pp
                    )

                return run

            def v_unit(mc):
                def run(ptag="pair"):
                    vp = ps.tile(
                        [128, H * KD], F32, tag=ptag,
                        bufs=3 if ptag == "pair" else 1, name="vp"
                    )
                    nc.tensor.matmul(
                        out=vp,
                        lhsT=ht_slice(b, mc * 128, (mc + 1) * 128),
                        rhs=wv_sb,
                        start=True,
                        stop=True,
                    )
                    vt = v_nat[mc]
                    nc.vector.tensor_copy(
                        out=vt[:, :, 0:KD],
                        in_=vp.rearrange("p (h k) -> p h k", k=KD),
                    )
                    nc.vector.memset(vt[:, :, KD : KD + 1], 1.0)
                    nc.vector.memset(vt[:, :, KD + 1 : VP], 0.0)

                return run

            if split_first:
                # pre-stream: only quad 0's nt=0 projections, so the first
                # compat+exp aren't queued behind quad 1's (whose wpk_b
                # weights land later).  k1h/v0/q1h burst into steps 0-1.
                first = [
                    proj_half(wk_q[0], kb[0], 0),
                    proj_half(wq_q[0], qb[0], 0),
                    v_unit(0),
                ]
                rest = [
                    proj_half(wk_q[1], kb[1], 0),
                    proj_half(wq_q[1], qb[1], 0),
                    v_unit(1),
                    v_unit(2),
                    v_unit(3),
                    proj_half(wk_q[0], kb[0], 1),
                    proj_half(wk_q[1], kb[1], 1),
                    v_unit(4),
                    v_unit(5),
                    v_unit(6),
                    v_unit(7),
                    proj_half(wq_q[0], qb[0], 1),
                    proj_half(wq_q[1], qb[1], 1),
                ]
                return first, rest
            return None, [
                proj_half(wk_q[0], kb[0], 0),
                proj_half(wq_q[0], qb[0], 0),
                proj_half(wk_q[1], kb[1], 0),
                proj_half(wq_q[1], qb[1], 0),
                v_unit(0),
                v_unit(1),
                v_unit(2),
                v_unit(3),
                proj_half(wk_q[0], kb[0], 1),
                proj_half(wk_q[1], kb[1], 1),
                v_unit(4),
                v_unit(5),
                v_unit(6),
                v_unit(7),
                proj_half(wq_q[0], qb[0], 1),
                proj_half(wq_q[1], qb[1], 1),
            ]

        def make_epilogue(b, nt, hp):
            """Normalize + W_out + residual for (b, nt) as staged closures.
            Stages are spaced so at most one borrows a pair PSUM buffer at
            a time, and every consumer fires >=2 steps after its producer."""
            h_sb = ios[b]["h"]
            hus, hn2, oms = {}, {}, {}
            st = {}

            dbg = DEBUG and b == 0 and nt == 0

            def hu_q(q):
                def run():
                    hu = sb.tile([128, NT], F32, tag=f"hu{q}", name=f"hu{q}")
                    nc.vector.tensor_copy(out=hu, in_=hp[q])
                    hus[q] = hu
                    if dbg:
                        nc.sync.dma_start(out=dbg_hu[q, :, :], in_=hu)
                return run

            def gather_q(q):
                def run():
                    d128 = sb.tile([128, NR], F32, tag=f"d128{q}", name=f"d1{q}")
                    for c in range(4):
                        srcp = hus[q][32 * c + KD : 32 * c + KD + 1, :]
                        src_r = bass.AP(
                            tensor=srcp.tensor,
                            offset=srcp.offset,
                            ap=[list(srcp.ap[0]), [NR, 32], [1, NR]],
                        )
                        nc.gpsimd.dma_start(
                            out=d128[32 * c : 32 * c + 32, :], in_=src_r
                        )
                    st[f"d128{q}"] = d128
                    if dbg:
                        nc.sync.dma_start(out=dbg_d[q, :, :], in_=d128)
                return run

            def recip_q(q):
                def run():
                    r128b = sb.tile([128, NR], BF16, tag=f"r128b{q}", name=f"rb{q}")
                    with nc.allow_low_precision(
                        reason="denominator recip to bf16 is within gate"
                    ):
                        nc.vector.reciprocal(out=r128b, in_=st[f"d128{q}"])
                    st[f"r128b{q}"] = r128b
                    if dbg:
                        nc.sync.dma_start(out=dbg_r[q, :, :], in_=r128b)
                return run

            def scatter_both():
                for q in range(2):
                    rec4 = sb.tile([4, NT], BF16, tag=f"rec4{q}", name=f"rc{q}")
                    dst = rec4[:, :]
                    dst_r = bass.AP(
                        tensor=dst.tensor,
                        offset=dst.offset,
                        ap=[list(dst.ap[0]), [NR, 32], [1, NR]],
                    )
                    nc.gpsimd.dma_start(out=dst_r, in_=st[f"r128b{q}"])
                    st[f"rec4{q}"] = rec4
                    if dbg:
                        nc.sync.dma_start(out=dbg_rec[q, :, :], in_=rec4)

            def selmm_both():
                bc_ps = ps.tile([128, 2, NT], F32, tag="pair", bufs=3, name="bc")
                for q in range(2):
                    nc.tensor.matmul(
                        out=bc_ps[:, q, :],
                        lhsT=sel_sb,
                        rhs=st[f"rec4{q}"],
                        start=True,
                        stop=True,
                    )
                st["bc"] = bc_ps

            def norm_q(q):
                def run():
                    hn = sb.tile([128, NT], BF16, tag=f"hn{q}", name=f"hn{q}")
                    nc.vector.tensor_mul(hn, hus[q], st["bc"][:, q, :])
                    hn2[q] = hn
                return run

            def out_mm_all():
                o_ps = ps.tile([128, 4, E], F32, tag="pair", bufs=3, name="op")
                for cl in range(4):
                    for q in range(2):
                        nc.tensor.matmul(
                            out=o_ps[:, cl, :],
                            lhsT=hn2[q][:, cl * 128 : (cl + 1) * 128],
                            rhs=wo_q[q],
                            start=(q == 0),
                            stop=(q == 1),
                        )
                st["o_ps"] = o_ps

            def out_fin_all():
                cc = nt * (NT // 128)
                ob = sb.tile([128, 4, E], F32, tag="ob", name="ob")
                nc.vector.tensor_add(ob, st["o_ps"], h_sb[:, cc : cc + 4, :])
                nc.sync.dma_start(
                    out=out_d[b, cc * 128 : (cc + 4) * 128, :].rearrange(
                        "(c p) e -> p c e", p=128
                    ),
                    in_=ob,
                )

            # spacing: at most one pair-PSUM borrow live at a time (the
            # merged bc until both norms, the merged o_ps until out_fin),
            # and every cross-engine consumer fires >=2 steps after its
            # producer so in-order queues never head-of-line block.
            stages = [
                (1, hu_q(0)),
                (3, hu_q(1)),
                (5, gather_q(0)),
                (7, gather_q(1)),
                (9, recip_q(0)),
                (10, recip_q(1)),
                (11, scatter_both),
                (13, selmm_both),
                (15, norm_q(0)),
                (17, norm_q(1)),
                (19, out_mm_all),
                (22, out_fin_all),
            ]
            return stages

        def make_tail_epilogue(b, nt, hp):
            """Latency-optimized final chain.  Quads finish in order 1 then
            0 (the final segment streams pairs 2,3,0,1).  Quad 1's hu copy
            rides the ScalarE (idle once exps end); reciprocals run as
            exp(-ln d) on ScalarE -- Ln and Exp share one ACT table set
            (natural_log_exp) so there is no table switch and the
            8-cyc/elem DVE iterative divide is avoided."""
            h_sb = ios[b]["h"]
            hus, den_ps, rec4s, rec_ps, hn2 = {}, {}, {}, {}, {}

            def hu_q(q, eng):
                def run():
                    hu = sb.tile([128, NT], BF16, tag=f"hut{q}", name=f"hut{q}")
                    if eng == "scalar":
                        nc.scalar.copy(out=hu, in_=hp[q])
                    else:
                        nc.vector.tensor_copy(out=hu, in_=hp[q])
                    hus[q] = hu
                return run

            def ext_q(q):
                def run():
                    d4 = ps.tile([4, NT], F32, tag="pair", bufs=3, name="d4")
                    nc.tensor.matmul(
                        out=d4, lhsT=sel4_sb, rhs=hus[q], start=True, stop=True
                    )
                    den_ps[q] = d4
                return run

            def recip_q(q):
                def run():
                    lnd = sb.tile([4, NT], F32, tag=f"lnd{q}", name=f"ln{q}")
                    nc.scalar.activation(
                        out=lnd,
                        in_=den_ps[q],
                        func=mybir.ActivationFunctionType.Ln,
                    )
                    rec4 = sb.tile([4, NT], BF16, tag=f"rect{q}", name=f"rt{q}")
                    with nc.allow_low_precision(
                        reason="denominator recip to bf16 is within gate"
                    ):
                        nc.scalar.activation(
                            out=rec4,
                            in_=lnd,
                            func=mybir.ActivationFunctionType.Exp,
                            scale=-1.0,
                        )
                    rec4s[q] = rec4
                return run

            def selmm_q(q):
                def run():
                    bc_ps = ps.tile([128, NT], F32, tag="pair", bufs=3, name="bc")
                    nc.tensor.matmul(
                        out=bc_ps, lhsT=sel_sb, rhs=rec4s[q], start=True, stop=True
                    )
                    rec_ps[q] = bc_ps
                return run

            def norm_q(q):
                def run():
                    hn = sb.tile([128, NT], BF16, tag=f"hn{q}", name=f"hn{q}")
                    nc.vector.tensor_mul(hn, hus[q], rec_ps[q])
                    hn2[q] = hn
                return run

            def out_cp(cp):
                def run():
                    cc = nt * (NT // 128) + cp * 2
                    o_ps = ps.tile([128, 2, E], F32, tag="pair", bufs=3, name="op")
                    for cl in range(2):
                        for q in range(2):
                            nc.tensor.matmul(
                                out=o_ps[:, cl, :],
                                lhsT=hn2[q][
                                    :,
                                    (cp * 2 + cl) * 128 : (cp * 2 + cl + 1) * 128,
                                ],
                                rhs=wo_q[q],
                                start=(q == 0),
                                stop=(q == 1),
                            )
                    ob = sb.tile([128, 2, E], F32, tag="ob", name="ob")
                    nc.vector.tensor_add(ob, o_ps, h_sb[:, cc : cc + 2, :])
                    nc.sync.dma_start(
                        out=out_d[b, cc * 128 : (cc + 2) * 128, :].rearrange(
                            "(c p) e -> p c e", p=128
                        ),
                        in_=ob,
                    )
                return run

            return {
                "hu": {q: hu_q(q, "scalar" if q == 1 else "vector") for q in range(2)},
                "ext": {q: ext_q(q) for q in range(2)},
                "recip": {q: recip_q(q) for q in range(2)},
                "selmm": {q: selmm_q(q) for q in range(2)},
                "norm": {q: norm_q(q) for q in range(2)},
                "out": [out_cp(0), out_cp(1)],
            }

        # ---- main stream over (batch, n-half) segments ----
        prefetch(0, first=True)
        b0_first, b0_rest = make_prologue_units(0, split_first=True)
        for u in b0_first:
            u()
        pending = None
        prologue_units = b0_rest
        unit_next = [0]
        units_fired = [0]
        unit_spacing = 2
        tail = None
        carry = []  # last 2 PVs of the previous segment (masks not yet done)
        for b in range(BPC):
            qb, kb, v_nat = bands.pop(b)
            for nt in range(N // NT):
                is_final = b == BPC - 1 and nt == N // NT - 1
                if nt == 1 and b + 1 < BPC:
                    prologue_units = (
                        prologue_units + make_prologue_units(b + 1)[1]
                    )
                    unit_spacing = 3
                pair_order = [2, 3, 0, 1] if is_final else [0, 1, 2, 3]
                pv_queue = []
                emit_pv = None

                step = 0
                for mc in range(MC):
                    for pair in pair_order:
                        quad, b0 = pair // 2, (pair * 2) % 4
                        cps = ps.tile(
                            [128, 2 * NT], F32, tag="pair", bufs=3, name="cp"
                        )
                        for j in range(2):
                            bd = b0 + j
                            nc.tensor.matmul(
                                out=cps[:, j * NT : (j + 1) * NT],
                                lhsT=kb[quad][
                                    32 * bd : 32 * bd + KD,
                                    mc * 128 : (mc + 1) * 128,
                                ],
                                rhs=qb[quad][
                                    32 * bd : 32 * bd + KD,
                                    nt * NT : (nt + 1) * NT,
                                ],
                                start=True,
                                stop=True,
                                tile_position=(32 * bd, 0),
                            )
                        if step == 5:
                            # hp accumulators allocated LAZILY at their
                            # first write (the first PV drains here): the
                            # banks stay free through steps 0-4 so early
                            # prologue units can borrow them instead of
                            # displacing the compat rotation slots, and
                            # all prior-generation reads (hu copies,
                            # carried PVs) are emitted before this point.
                            hp = [
                                ps.tile(
                                    [128, NT], F32, tag=f"hp{q}", bufs=1,
                                    name=f"hp{q}"
                                )
                                for q in range(2)
                            ]

                            def emit_pv(pm, mc, pair, hp=hp, v_nat=v_nat):
                                for j in range(2):
                                    hh = pair * 2 + j
                                    c = hh % 4
                                    nc.tensor.matmul(
                                        out=hp[hh // 4][32 * c : 32 * c + VP, :],
                                        lhsT=v_nat[mc][:, hh, :],
                                        rhs=pm[:, j * NT : (j + 1) * NT],
                                        start=(mc == 0),
                                        stop=(mc == MC - 1),
                                        tile_position=(0, 32 * c),
                                    )

                            if is_final:
                                tail = make_tail_epilogue(b, nt, hp)
                            else:
                                epi = make_epilogue(b, nt, hp)
                        if carry and step <= 1:
                            f, cpm, cmc, cpair = carry.pop(0)
                            f(cpm, cmc, cpair)
                        if is_final and step >= 29:
                            lag = 2 if step == 29 else 1
                        else:
                            lag = PV_LAG
                        while len(pv_queue) > lag:
                            emit_pv(*pv_queue.pop(0))
                        if b == 0 and nt == 0:
                            deferred_b0(step)
                            if step == 0:
                                prologue_units.pop(0)("hp0")  # k1h nt0
                                units_fired[0] += 1
                                unit_next[0] = 1
                            elif step == 1:
                                prologue_units.pop(0)("hp1")  # q1h nt0
                                units_fired[0] += 1
                                unit_next[0] = 3
                        if nt == 0 and b + 1 < BPC and step == 4:
                            prefetch(b + 1)
                        stage_fired = False
                        if pending is not None and pending and step == pending[0][0]:
                            pending.pop(0)[1]()
                            stage_fired = True
                            if not pending:
                                pending = None
                        if (
                            not stage_fired
                            and prologue_units
                            and (step >= 4 or (b == 0 and nt == 0))
                            and step >= unit_next[0]
                        ):
                            if step <= 4 and b == 0 and nt == 0:
                                ptag = ("hp0", "hp1")[step % 2]
                            elif step == 4:
                                ptag = "hp1"  # hp1 idle: hu1 read was step 3
                            else:
                                ptag = "pair"
                            prologue_units.pop(0)(ptag)
                            units_fired[0] += 1
                            # tighten near segment end so the last units'
                            # pool-slot displacement doesn't land on the
                            # next segment's first compats
                            if units_fired[0] <= 4:
                                sp = 2
                            elif len(prologue_units) <= 2 and step >= 20:
                                sp = 2
                            else:
                                sp = 3
                            unit_next[0] = step + max(sp, unit_spacing)
                        p_sb = sb.tile([128, 2 * NT], BF16, tag="p", bufs=8, name="p")
                        nc.scalar.activation(
                            out=p_sb,
                            in_=cps,
                            func=mybir.ActivationFunctionType.Exp,
                            scale=0.25,
                        )
                        pm = sb.tile([128, 2 * NT], BF16, tag="pm", bufs=8, name="pm")
                        # stride-0 repeat of the adj row block across the two
                        # heads; adjT tiles are single-writer (one DMA each)
                        # so the raw AP's coarse dependency attribution is
                        # safe.
                        adj_src = adj_of(b, mc, nt)
                        adj_rep = bass.AP(
                            tensor=adj_src.tensor,
                            offset=adj_src.offset,
                            ap=[list(adj_src.ap[0]), [0, 2]]
                            + [list(a) for a in adj_src.ap[1:]],
                        )
                        nc.vector.tensor_mul(pm, p_sb, adj_rep)
                        if DEBUG and b == 0 and nt == 0 and step == 0:
                            nc.sync.dma_start(out=dbg_pm[:, :], in_=pm)
                        pv_queue.append((pm, mc, pair))
                        step += 1
                if is_final:
                    # drain with quad chains interleaved: pv_queue holds
                    # pairs 0 (step 30) and 1 (step 31) = quad 0.  Quad 1
                    # (pairs 2,3, PVs already emitted) chains on the idle
                    # ScalarE while quad 0's last PV waits on the last mask.
                    tail["hu"][1]()  # ScalarE, right after the last exp
                    emit_pv(*pv_queue.pop(0))  # PV pair 0
                    tail["ext"][1]()
                    emit_pv(*pv_queue.pop(0))  # PV pair 1 (waits last mask)
                    tail["recip"][1]()  # ScalarE: ln, exp(-x)
                    tail["hu"][0]()  # DVE, needs PV pair 1
                    tail["ext"][0]()
                    tail["recip"][0]()
                    tail["selmm"][1]()
                    tail["norm"][1]()
                    tail["selmm"][0]()
                    tail["norm"][0]()
                    tail["out"][0]()
                    tail["out"][1]()
                else:
                    # flush all but the last two PVs: their masks finish
                    # only after the segment's last exps, so emitting them
                    # here would block the PE ahead of the next segment's
                    # first compats.  They fire at the next steps 0/1.
                    while len(pv_queue) > 2:
                        emit_pv(*pv_queue.pop(0))
                    carry = [(emit_pv, pm_, mc_, pair_) for pm_, mc_, pair_ in pv_queue]
                    pv_queue = []
                    pending = epi
                unit_next[0] = 0
            del ios[b]
    return nc


def _split_multi_waits(nc):
    """walrus codegen in this container allows only one sync-wait per
    instruction; hoist extra waits onto preceding same-engine nops."""
    import copy
    import bass_rust

    tmpl_nc = bass.Bass()
    tmpls = {}
    for en in ["vector", "scalar", "tensor", "gpsimd", "sync"]:
        ins = getattr(tmpl_nc, en).nop().ins
        tmpls[str(ins.engine)] = ins

    uid = [0]
    for fn in nc.m.functions:
        for bb in fn.blocks:
            out = []
            for ins in bb.instructions:
                si = ins.sync_info
                waits = list(si.on_wait) if si is not None else []
                if len(waits) > 1:
                    for w in waits[:-1]:
                        nop = copy.deepcopy(tmpls[str(ins.engine)])
                        uid[0] += 1
                        nop.name = f"I-splitw-{uid[0]}"
                        nop.sync_info = bass_rust.SyncInfo(
                            on_wait=[w], on_update=[]
                        )
                        out.append(nop)
                    ins.sync_info = bass_rust.SyncInfo(
                        on_wait=[waits[-1]], on_update=list(si.on_update)
                    )
                out.append(ins)
            bb.instructions = out
    return nc


_cache = {}


def _get_nc():
    if "nc" not in _cache:
        _cache["nc"] = _split_multi_waits(build_kernel())
    return _cache["nc"]


def _prep_weights(W_query, W_key, W_val, W_out):
    bf = ml_dtypes.bfloat16
    wqb = np.zeros((2, E, 128), bf)
    wkb = np.zeros((2, E, 128), bf)
    wob = np.zeros((2, 128, E), bf)
    for h in range(H):
        q, c = h // 4, h % 4
        wqb[q, :, 32 * c : 32 * c + KD] = W_query[h].astype(bf)
        wkb[q, :, 32 * c : 32 * c + KD] = W_key[h].astype(bf)
        wob[q, 32 * c : 32 * c + KD, :] = W_out[h].astype(bf)
    wv = np.ascontiguousarray(
        np.asarray(W_val, np.float32).transpose(1, 0, 2).reshape(E, H * KD)
    ).astype(bf)
    ext = np.zeros((128, 128), bf)
    for c in range(4):
        ext[32 * c + KD, c] = 1.0  # sel4: extract den row of band c
    wpk = np.concatenate(
        [wkb[0], wqb[0], wkb[1], wqb[1], wv, wob[0], wob[1], ext], axis=1
    )
    sel = np.zeros((4, 128), ml_dtypes.bfloat16)
    for c in range(4):
        sel[c, 32 * c : 32 * c + KD + 1] = 1.0
    return np.ascontiguousarray(wpk), sel


def kernel(h, adj_c, W_query, W_key, W_val, W_out, trace=False):
    h = np.asarray(h, np.float32)
    adj = np.asarray(adj_c)
    bf = ml_dtypes.bfloat16
    hT = np.ascontiguousarray(h.transpose(0, 2, 1)).astype(bf)  # [B, E, N]
    # partition-contiguous layouts: one DMA descriptor per partition
    adjT = np.ascontiguousarray(
        adj.transpose(0, 2, 1).astype(bf).reshape(B, MC, 128, N).transpose(0, 2, 1, 3)
    )  # [B, 128, MC, N] bf16
    h_r = np.ascontiguousarray(
        h.reshape(B, MC, 128, E).transpose(0, 2, 1, 3)
    )  # [B, 128, MC, E]
    wpk, sel = _prep_weights(
        np.asarray(W_query, np.float32),
        np.asarray(W_key, np.float32),
        np.asarray(W_val, np.float32),
        np.asarray(W_out, np.float32),
    )

    nc = _get_nc()
    in_maps = []
    for c in range(CORES):
        s = slice(c * BPC, (c + 1) * BPC)
        in_maps.append(
            {
                "ht": np.ascontiguousarray(hT[s]),
                "hn": np.ascontiguousarray(h_r[s]),
                "adjt": np.ascontiguousarray(adjT[s]),
                "wpk": wpk,
                "sel": sel,
            }
        )
    res = run_bass_kernel_spmd(nc, in_maps, core_ids=list(range(CORES)), trace=trace)
    out = np.concatenate([r["out"] for r in res.results], axis=0)
    if trace:
        return out, res
    return out
